# revision 37
# baseline (speedup 1.0000x reference)
"""Trainium2 Bass kernel for the Deter GRU-MLP block (RSSM deter update).

Sharding: data-parallel over batch B=4096 across 8 NeuronCores (512 rows
each), all parameters replicated; no collectives.

Design (mixed fp8/bf16, measured 215us TimelineSim vs 407us baseline,
hardware-validated rel-max err 1.42e-2):
- Activations live transposed in SBUF (features on partitions, batch on
  the 512-wide free axis); weights consumed in natural [K, M] layout.
- Big GEMMs run as fp8(e4m3) DoubleRow matmuls (0.5 cyc/row, two k-tiles
  per instruction, host-packed [pairs, 128, 2, M] weights): branch W0/W1,
  both slices of the block-diagonal L0, and the GRU gate GEMM.  L1 stays
  bf16 for accuracy (numpy quantization model: fp8_sim3.py).
- All weights are scaled by 64 on the host (fp8 normal range) with norm
  gains folded in, so pre-norm PSUM values are 64*y.  The rmsnorm absorbs
  the scale: squares are (64y)^2 in bf16, the sqrt fold yields
  1/(64*rms) directly, and the gate nonlinearities use the ACT scale
  operand (sigmoid(acc/64 + b)).
- Per 4-tile norm unit: PSUM->SBUF copies ride the otherwise-idle ACT
  engine (Identity+bias) in L0/L1, squares are a 2x-rate bf16 DVE
  multiply, the sum-of-squares reduction is a ones-vector matmul, and
  silu is a batched ACT sigmoid + DVE multiply casting to fp8/bf16 on
  write.  The four branch norms share two fused sqrt/recip/broadcasts.
- Engines are in-order, so loops are software-pipelined: block g+1's
  normalize+silu is emitted during block g's GEMMs, each block's ss
  matmuls are deferred past the next block's GEMMs, and the first/last
  blocks around each norm barrier run per-tile (or per-half for the gate
  sigmoid/stt/tanh chain) to shorten restart and drain latency.
  Cross-phase weight prefetch tiles live in enclosing pool scopes so
  their DMAs do not inherit false WARs from reused SBUF regions.
- deter is read once as fp8 (branch-0 GEMM + L0 dg slice, resident) and
  once as bf16 (GRU mix operand, streamed during the gates phase);
  output is written bf16 and upcast on the host.
- Hardware-legality notes baked in: gpsimd(Pool) cannot touch PSUM and
  cannot run TensorScalarPtr; DoubleRow Ldweights rejects degenerate
  single-column stationary tiles; Sqrt lives in a different ACT table
  set than Sigmoid/Tanh, so table switches are pre-triggered by tiny
  warm-up ops off the critical path.
"""

import os
import sys
from contextlib import ExitStack

import numpy as np
import ml_dtypes as _ml

for _p in ("/opt/trn_rl_repo", "/opt/pypackages"):
    if os.path.isdir(_p) and _p not in sys.path:
        sys.path.insert(0, _p)

os.environ.setdefault("MYCRO_LOCAL_CACHE", "1")

import concourse.bass as bass  # noqa: E402
import concourse.bacc as bacc  # noqa: E402
import concourse.mybir as mybir  # noqa: E402
import concourse.tile as tile  # noqa: E402

# ---- problem constants (hardcoded; kernel.py must be self-contained) ----
P = 128
B = 4096
NCORES = 8
BC = B // NCORES  # 512 batch columns per core
DETER = 4096
STOCH = 1024
ACT_DIM = 32
DEMB = 16
HIDDEN = 512
BLOCKS = 8
OUT_B = DETER // BLOCKS  # 512
IN_B0 = 4 * HIDDEN + OUT_B  # 2560
EPS = 1e-4

ND = DETER // P       # 32 deter k/n tiles
NX = 4 * HIDDEN // P  # 16 x k tiles
WS = 64.0             # weight scale folded into rmsnorm / gate scales

# const-block column layout (single [P, C_NCOL] f32 DRAM input)
C_BX64 = 0            # 16: 64*(branch bias * gain)
C_BH064 = 16          # 32: 64*(bh0 * gh0)
C_BH164 = 48          # 32: 64*(bh1 * gh1)
C_BG = 80             # 96: bg (unscaled, sigmoid bias)
C_BG64 = 176          # 96: 64*bg (cand stt bias)
C_BGM1 = 272          # 96: bg - 1 (update sigmoid bias)
C_EPSH = 368          # 1: 4096*EPS
C_NEG1 = 369          # 1: -1.0 (update-gate bias)
C_NCOL = 370

f32 = mybir.dt.float32
f32r = mybir.dt.float32r
bf16 = mybir.dt.bfloat16
fp8 = mybir.dt.float8e4

# (L1 is the only bf16 GEMM; see fp8_sim3.py for the quantization study)

_PROG = {}


def _r(ap):
    return ap.bitcast(f32r)


def _build_program(zb):
    """Build the single-core SPMD Bass program (same on all 8 cores).

    zb: gate biases are all zero -> skip the bias K-row in the gate GEMM
    (saves a weight pair per m-tile and 3MB of DMA per core)."""
    AF = mybir.ActivationFunctionType
    Alu = mybir.AluOpType
    DR = mybir.MatmulPerfMode.DoubleRow
    nc = bacc.Bacc(trn_type="TRN2", target_bir_lowering=False, debug=False)

    def din(name, shape, dt=f32):
        return nc.dram_tensor(name, list(shape), dt, kind="ExternalInput").ap()

    dT8 = din("dT8", (DETER, BC), fp8)
    dTb = din("dTb", (DETER, BC), bf16)
    sT8 = din("sT8", (STOCH, BC), fp8)
    aT = din("aT", (ACT_DIM, BC))
    eT = din("eT", (DEMB, BC))
    W0p = din("W0p", (16, P, 2, HIDDEN), fp8)
    W1p = din("W1p", (P, 4, 2, HIDDEN), fp8)
    W2 = din("W2", (ACT_DIM, HIDDEN))
    W3 = din("W3", (DEMB, HIDDEN))
    Wh0x = din("Wh0x", (BLOCKS, P, 8, 2, OUT_B), fp8)
    Wh0d = din("Wh0d", (BLOCKS, P, 2, 2, OUT_B), fp8)
    Wh1 = din("Wh1", (BLOCKS, P, 4, OUT_B), bf16)
    NKP = 2 if zb else 3
    Wgp = din("Wgp", (BLOCKS, P, NKP, 2, 3 * OUT_B), fp8)
    cst = din("cst", (P, C_NCOL))
    cst8 = din("cst8", (P, 2 + 2 * BC), fp8)
    outT = nc.dram_tensor("outT", [DETER, BC], bf16,
                          kind="ExternalOutput").ap()

    with tile.TileContext(nc) as tc, ExitStack() as top:
        consts = top.enter_context(tc.tile_pool(name="consts", bufs=1))
        cst_sb = consts.tile([P, C_NCOL], f32)
        nc.sync.dma_start(out=_r(cst_sb), in_=_r(cst))
        cst8_sb = consts.tile([P, 2 + 2 * BC], fp8)
        nc.sync.dma_start(out=cst8_sb, in_=cst8)
        ones8 = cst8_sb[:, 0:2]
        ones_b16 = consts.tile([P, 1], bf16)
        nc.vector.memset(ones_b16, 1.0)
        onesp = cst8_sb[:, 2:2 + 2 * BC].rearrange("p (j b) -> p j b", j=2)
        bgm1 = cst_sb[:, C_BGM1:C_BGM1 + 96]
        epsh = cst_sb[:1, C_EPSH:C_EPSH + 1]
        neg1 = cst_sb[:, C_NEG1:C_NEG1 + 1]

        # resident regions
        mainp = top.enter_context(tc.tile_pool(name="mainp", bufs=1))
        main_sb = mainp.tile([P, ND, BC], bf16)   # pre-norm y64 / h0n
        dtbp = top.enter_context(tc.tile_pool(name="dtbp", bufs=1))
        dtb_sb = dtbp.tile([P, ND, BC], bf16)     # deter bf16 (L0 dg + mix)
        x8p = top.enter_context(tc.tile_pool(name="x8p", bufs=1))
        x8_sb = x8p.tile([P, NX, BC], fp8)        # branch outputs (L0 rhs)
        h1p = top.enter_context(tc.tile_pool(name="h1p", bufs=1))
        h1n8 = h1p.tile([P, ND, BC], fp8)         # L1 normalized (gates rhs)

        ysqp = top.enter_context(tc.tile_pool(name="ysqp", bufs=3))
        invp = top.enter_context(tc.tile_pool(name="invp", bufs=2))
        invbp = top.enter_context(tc.tile_pool(name="invbp", bufs=2))
        sigp = top.enter_context(tc.tile_pool(name="sigp", bufs=3))

        def act_warm(func, name):
            """Trigger an ACT table switch off the critical path."""
            t = invp.tile([1, 1], f32, tag="warm", name=f"warm_{name}")
            nc.scalar.activation(out=t, in_=epsh, func=func)

        def finish_norm(ss_flat, D, width, name):
            """invb = 1/(64*sqrt(ss/D + eps)), bf16, broadcast to all
            partitions.  ss_flat: [1, width*BC] (PSUM)."""
            sq = invp.tile([1, width * BC], f32, tag="sq", name=f"sq_{name}")
            nc.scalar.activation(out=sq, in_=ss_flat, func=AF.Sqrt,
                                 bias=epsh, scale=1.0 / D)
            act_warm(AF.Sigmoid, f"sg_{name}")  # reload hidden under recip
            inv1 = invp.tile([1, width * BC], bf16, tag="inv1",
                             name=f"inv1_{name}")
            with nc.allow_low_precision(reason="bf16 rstd broadcast"):
                nc.vector.reciprocal(inv1, sq)
            invb = invbp.tile([P, width * BC], bf16, tag="invb",
                              name=f"invb_{name}")
            nc.gpsimd.partition_broadcast(invb, inv1)
            return invb

        def norm_silu_unit(unit_y, invb, out_unit, name, per_tile=False,
                           mul_pool=False):
            """out_unit <- silu(unit_y * invb) = t*sigmoid(t), t=y*inv.
            per_tile=True pipelines at tile granularity (lower latency
            right after a norm barrier)."""
            mul_eng = nc.gpsimd if mul_pool else nc.vector
            for m in range(4):
                nc.vector.tensor_mul(unit_y[:, m, :], unit_y[:, m, :],
                                     invb[:, m * BC:(m + 1) * BC]
                                     if invb.shape[-1] == 4 * BC else invb)
            s = sigp.tile([P, 4, BC], bf16, tag="sig", name=f"sig_{name}")
            if per_tile:
                for m in range(4):
                    nc.scalar.activation(out=s[:, m, :], in_=unit_y[:, m, :],
                                         func=AF.Sigmoid)
                    mul_eng.tensor_mul(out_unit[:, m, :], unit_y[:, m, :],
                                       s[:, m, :])
            else:
                nc.scalar.activation(
                    out=s.rearrange("p a b -> p (a b)"),
                    in_=unit_y.rearrange("p a b -> p (a b)"), func=AF.Sigmoid)
                mul_eng.tensor_mul(
                    out_unit.rearrange("p a b -> p (a b)"),
                    unit_y.rearrange("p a b -> p (a b)"),
                    s.rearrange("p a b -> p (a b)"))

        # ============ phases A, L0, L1 (shared PSUM layout) ============
        with ExitStack() as ph_al:
            psum_acc = ph_al.enter_context(
                tc.tile_pool(name="pacc", bufs=5, space="PSUM"))
            psum_ss = ph_al.enter_context(
                tc.tile_pool(name="pss", bufs=1, space="PSUM"))
            d8pool = ph_al.enter_context(tc.tile_pool(name="d8r", bufs=1))
            d8sb = d8pool.tile([P, ND, BC], fp8)  # fp8 deter (br0 + L0 dg)

            def unit_post(unit_y, accs, b64col0, ss, ss_first, ss_last,
                          name, act_copy, sq_pool=False):
                """copy accs (+64*bias) into unit_y (bf16) -- on ACT
                (Identity) when act_copy else DVE; squares (64y)^2 bf16.
                Returns a closure emitting the 4 ss ones-matmuls, so the
                caller can defer them past the next block's GEMMs (the PE
                is in-order; immediate ss would stall it on this block's
                elementwise post)."""
                for m in range(4):
                    bcol = cst_sb[:, b64col0 + m:b64col0 + m + 1]
                    if act_copy and (act_copy == 2 or m % 2 == 0):
                        nc.scalar.activation(out=unit_y[:, m, :],
                                             in_=accs[m], func=AF.Identity,
                                             bias=bcol)
                    else:
                        nc.vector.tensor_scalar_add(unit_y[:, m, :],
                                                    accs[m], bcol)
                ysq = ysqp.tile([P, 4, BC], bf16, tag="ysq",
                                name=f"ysq_{name}")
                if sq_pool:
                    for half in range(2):
                        seg = unit_y[:, 2 * half:2 * half + 2, :].rearrange(
                            "p a b -> p (a b)")
                        nc.gpsimd.tensor_mul(
                            ysq[:, 2 * half:2 * half + 2, :].rearrange(
                                "p a b -> p (a b)"), seg, seg)
                else:
                    for t in range(4):
                        nc.vector.tensor_mul(ysq[:, t, :], unit_y[:, t, :],
                                             unit_y[:, t, :])

                def emit_ss():
                    for t in range(4):
                        nc.tensor.matmul(
                            ss, lhsT=ones_b16, rhs=ysq[:, t, :],
                            start=(ss_first and t == 0),
                            stop=(ss_last and t == 3))
                return emit_ss

            # ---------------- phase A: four input branches ----------------
            with ExitStack() as ph_a:
                w0p_pool = ph_a.enter_context(
                    tc.tile_pool(name="w0p", bufs=8))
                sp = ph_a.enter_context(tc.tile_pool(name="sp", bufs=1))

                sT_sb = sp.tile([P, STOCH // P, BC], fp8)
                aT_sb = sp.tile([ACT_DIM, BC], f32)
                eT_sb = sp.tile([DEMB, BC], f32)
                an_sb = sp.tile([ACT_DIM, BC], f32)
                w3t = sp.tile([DEMB, HIDDEN], f32)
                w2t = sp.tile([ACT_DIM, HIDDEN], f32)
                w1t = sp.tile([P, 4, 2, HIDDEN], fp8)

                # the 4MB branch-0 stream is the phase-A long pole: first
                w0_slabs = []
                for c in range(8):
                    nc.sync.dma_start(
                        out=d8sb[:, 4 * c:4 * c + 4, :],
                        in_=dT8[512 * c:512 * (c + 1), :].rearrange(
                            "(s p) b -> p s b", p=P))
                    w0 = w0p_pool.tile([P, 2, 2, HIDDEN], fp8, tag="w0",
                                       name=f"w0_{c}")
                    nc.sync.dma_start(
                        out=w0, in_=W0p[2 * c:2 * c + 2].rearrange(
                            "s p j m -> p s j m"))
                    w0_slabs.append(w0)
                    if c == 0:
                        nc.sync.dma_start(out=_r(eT_sb), in_=_r(eT))
                        nc.sync.dma_start(out=_r(w3t), in_=_r(W3))
                        nc.sync.dma_start(out=aT_sb, in_=aT)
                        nc.sync.dma_start(out=_r(w2t), in_=_r(W2))
                    if c == 1:
                        nc.sync.dma_start(
                            out=sT_sb,
                            in_=sT8.rearrange("(s p) b -> p s b", p=P))
                        nc.sync.dma_start(out=w1t, in_=W1p)

                # action preprocess: a / max(|a|, 1)
                ab = sp.tile([ACT_DIM, BC], f32)
                nc.scalar.activation(out=ab, in_=aT_sb, func=AF.Abs)
                act_warm(AF.Sqrt, "br")
                nc.vector.tensor_scalar_max(ab, ab, 1.0)
                nc.vector.reciprocal(ab, ab)
                nc.vector.tensor_mul(_r(an_sb), aT_sb, ab)

                def branch_small(br, wt, rhs):
                    accs = []
                    for m in range(4):
                        acc = psum_acc.tile([P, BC], f32, tag="acc",
                                            name=f"acc_br{br}_{m}")
                        nc.tensor.matmul(acc,
                                         lhsT=_r(wt[:, m * P:(m + 1) * P]),
                                         rhs=_r(rhs), start=True, stop=True)
                        accs.append(acc)
                    return accs

                def branch_dr(br, npairs, wslab, rhs_pair):
                    accs = [psum_acc.tile([P, BC], f32, tag="acc",
                                          name=f"acc_br{br}_{m}")
                            for m in range(4)]
                    for kp in range(npairs):
                        w = wslab(kp)
                        rhs = rhs_pair(kp)
                        for m in range(4):
                            nc.tensor.matmul(
                                accs[m], lhsT=w[:, :, m * P:(m + 1) * P],
                                rhs=rhs, start=(kp == 0),
                                stop=(kp == npairs - 1), perf_mode=DR)
                    return accs

                # branches 1-3 share one fused norm (single sqrt/recip/
                # broadcast over 3 stacked ss slots); branch 0 -- whose GEMM
                # is gated on the big dT8/W0p DMA stream -- normalizes on
                # its own so the other three silus overlap that stream.
                ss3 = psum_ss.tile([1, 2, BC], f32, tag="ss3", name="ss3")

                def branch_post(br, accs, ss):
                    unit = main_sb[:, 4 * br:4 * br + 4, :]
                    unit_post(unit, accs, C_BX64 + 4 * br, ss,
                              True, True, f"br{br}", act_copy=2,
                              sq_pool=(br == 3))()

                branch_post(3, branch_small(3, w3t, eT_sb), ss3[:, 0, :])
                branch_post(2, branch_small(2, w2t, an_sb), ss3[:, 1, :])
                ssb1 = psum_ss.tile([1, BC], f32, tag="ss", name="ssb1")
                branch_post(1, branch_dr(
                    1, 4, lambda kp: w1t[:, kp, :, :],
                    lambda kp: sT_sb[:, 2 * kp:2 * kp + 2, :]), ssb1)
                invb3 = finish_norm(ss3.rearrange("o a b -> o (a b)"),
                                    HIDDEN, 2, "br32")
                for i, br in enumerate((3, 2)):
                    norm_silu_unit(
                        main_sb[:, 4 * br:4 * br + 4, :],
                        invb3[:, i * BC:(i + 1) * BC],
                        x8_sb[:, 4 * br:4 * br + 4, :], f"br{br}",
                        mul_pool=(br == 3))
                invb1b = finish_norm(ssb1, HIDDEN, 1, "br1")
                norm_silu_unit(main_sb[:, 4:8, :], invb1b,
                               x8_sb[:, 4:8, :], "br1")

                ssb0 = psum_ss.tile([1, BC], f32, tag="ss", name="ssb0")
                branch_post(0, branch_dr(
                    0, 16, lambda kp: w0_slabs[kp // 2][:, kp % 2, :, :],
                    lambda kp: d8sb[:, 2 * kp:2 * kp + 2, :]), ssb0)
                invb0b = finish_norm(ssb0, HIDDEN, 1, "br0")
                norm_silu_unit(main_sb[:, 0:4, :], invb0b,
                               x8_sb[:, 0:4, :], "br0", per_tile=True)

            # ---------- L0: BlockLinear(2560 -> 512/block) ----------
            with ExitStack() as ph_l:
                wh0xp = ph_l.enter_context(
                    tc.tile_pool(name="wh0xp", bufs=2))
                wh0dp = ph_l.enter_context(
                    tc.tile_pool(name="wh0dp", bufs=2))
                wh1p = ph_l.enter_context(tc.tile_pool(name="wh1p", bufs=3))

                ss0 = psum_ss.tile([1, BC], f32, tag="ss", name="ss_l0")
                pend0 = []
                for g in range(BLOCKS):
                    wd = wh0dp.tile([P, 2, 2, OUT_B], fp8, tag="wh0d",
                                    name=f"wh0d_{g}")
                    nc.sync.dma_start(out=wd, in_=Wh0d[g])
                    wx = wh0xp.tile([P, 8, 2, OUT_B], fp8, tag="wh0x",
                                    name=f"wh0x_{g}")
                    nc.sync.dma_start(out=wx, in_=Wh0x[g])
                    accs = [psum_acc.tile([P, BC], f32, tag="acc",
                                          name=f"acc_h0_{g}_{m}")
                            for m in range(4)]
                    # dg (fp8 DR) matmuls for all m first: they only need
                    # the resident fp8 deter + wd, so the PE can run them
                    # while the branch silu (x8) is still finishing
                    for m in range(4):
                        for kp in range(2):
                            nc.tensor.matmul(
                                accs[m],
                                lhsT=wd[:, kp, :, m * P:(m + 1) * P],
                                rhs=d8sb[:, 4 * g + 2 * kp:4 * g + 2 * kp + 2,
                                         :],
                                start=(kp == 0), stop=False, perf_mode=DR)
                    for m in range(4):
                        for kp in (2, 3, 4, 5, 6, 7, 0, 1):
                            nc.tensor.matmul(
                                accs[m], lhsT=wx[:, kp, :, m * P:(m + 1) * P],
                                rhs=x8_sb[:, 2 * kp:2 * kp + 2, :],
                                start=False, stop=(kp == 1), perf_mode=DR)
                    # deferred ss for the previous block: keeps the PE from
                    # stalling on this block's elementwise post
                    if pend0:
                        pend0.pop()()
                    pend0.append(unit_post(
                        main_sb[:, 4 * g:4 * g + 4, :], accs,
                        C_BH064 + 4 * g, ss0, g == 0, g == BLOCKS - 1,
                        f"l0_{g}", act_copy=True, sq_pool=(g % 2 == 0)))
                    if g == 5:
                        act_warm(AF.Sqrt, "l0")
                pend0.pop()()
                invb0 = finish_norm(ss0, DETER, 1, "l0")

                # --------- L1 (bf16), interleaved with the L0 norm ---------
                ss1 = psum_ss.tile([1, BC], f32, tag="ss", name="ss_l1")
                # h0n = silu(norm(h0)) in place; block 0 primed per-tile,
                # block g+1's silu is emitted during block g (in-order
                # engines would otherwise serialize consecutive blocks)
                norm_silu_unit(main_sb[:, 0:4, :], invb0,
                               main_sb[:, 0:4, :], "l1_0", per_tile=True)
                pend1 = []
                for g in range(BLOCKS):
                    unit = main_sb[:, 4 * g:4 * g + 4, :]
                    # stream the bf16 deter (GRU mix operand) here, where
                    # the DMA engines are otherwise idle
                    nc.sync.dma_start(
                        out=dtb_sb[:, 4 * g:4 * g + 4, :],
                        in_=dTb[512 * g:512 * (g + 1), :].rearrange(
                            "(s p) b -> p s b", p=P))
                    w1h = wh1p.tile([P, 4, OUT_B], bf16, tag="wh1",
                                    name=f"wh1_{g}")
                    nc.sync.dma_start(out=w1h, in_=Wh1[g])
                    accs = [psum_acc.tile([P, BC], f32, tag="acc",
                                          name=f"acc_h1_{g}_{m}")
                            for m in range(4)]
                    for m in range(4):
                        for kk in range(4):
                            nc.tensor.matmul(
                                accs[m], lhsT=w1h[:, kk, m * P:(m + 1) * P],
                                rhs=unit[:, kk, :],
                                start=(kk == 0), stop=(kk == 3))
                    if g + 1 < BLOCKS:
                        nxt = main_sb[:, 4 * (g + 1):4 * (g + 1) + 4, :]
                        norm_silu_unit(nxt, invb0, nxt, f"l1_{g + 1}")
                    if pend1:
                        pend1.pop()()
                    pend1.append(unit_post(
                        unit, accs, C_BH164 + 4 * g, ss1,
                        g == 0, g == BLOCKS - 1, f"l1_{g}",
                        act_copy=True, sq_pool=(g % 2 == 0)))
                    if g == 5:
                        act_warm(AF.Sqrt, "l1")
                pend1.pop()()
                invb1 = finish_norm(ss1, DETER, 1, "l1")

        # ------------- GRU gates + final mix (per block) -------------
        with ExitStack() as ph_g:
            gpsum = ph_g.enter_context(
                tc.tile_pool(name="gpsum", bufs=2, space="PSUM"))
            wgp = ph_g.enter_context(tc.tile_pool(name="wgpool", bufs=3))
            grup = ph_g.enter_context(tc.tile_pool(name="grup", bufs=3))

            norm_silu_unit(main_sb[:, 0:4, :], invb1, h1n8[:, 0:4, :],
                           "g0", per_tile=True)
            for g in range(BLOCKS):
                wg = wgp.tile([P, NKP, 2, 3 * OUT_B], fp8, tag="wg",
                              name=f"wg_{g}")
                nc.sync.dma_start(out=wg, in_=Wgp[g])
                r_sb = grup.tile([P, 4, BC], bf16, tag="r", name=f"r_{g}")
                c_sb = grup.tile([P, 4, BC], bf16, tag="c", name=f"c_{g}")
                u_sb = grup.tile([P, 4, BC], bf16, tag="u", name=f"u_{g}")

                def gate_group(grp):
                    """12 DoubleRow matmuls (2 data pairs + bias pair) into
                    a 4-bank PSUM group for gate third grp."""
                    acc4 = gpsum.tile([P, 4, BC], f32, tag="g4",
                                      name=f"acc_g{g}_{grp}")
                    for m in range(4):
                        mm = 4 * grp + m
                        for kp in range(NKP):
                            rhs = (onesp if kp == 2 else
                                   h1n8[:, 4 * g + 2 * kp:4 * g + 2 * kp + 2, :])
                            nc.tensor.matmul(
                                acc4[:, m, :],
                                lhsT=wg[:, kp, :, mm * P:(mm + 1) * P],
                                rhs=rhs, start=(kp == 0),
                                stop=(kp == NKP - 1), perf_mode=DR)
                    return acc4

                accr = gate_group(0)
                if g == BLOCKS - 1:
                    for m in range(4):
                        nc.scalar.activation(
                            out=r_sb[:, m, :], in_=accr[:, m, :],
                            func=AF.Sigmoid, scale=1.0 / WS)
                else:
                    for h in range(2):
                        nc.scalar.activation(
                            out=r_sb[:, 2 * h:2 * h + 2, :].rearrange(
                                "p a b -> p (a b)"),
                            in_=accr[:, 2 * h:2 * h + 2, :].rearrange(
                                "p a b -> p (a b)"),
                            func=AF.Sigmoid, scale=1.0 / WS)
                if g + 1 < BLOCKS:
                    nxt = main_sb[:, 4 * (g + 1):4 * (g + 1) + 4, :]
                    norm_silu_unit(nxt, invb1,
                                   h1n8[:, 4 * (g + 1):4 * (g + 1) + 4, :],
                                   f"g{g + 1}", mul_pool=False)
                accc = gate_group(1)
                if g == BLOCKS - 1:
                    for m in range(4):
                        nc.vector.scalar_tensor_tensor(
                            out=c_sb[:, m, :], in0=accc[:, m, :],
                            scalar=1.0 / WS, in1=r_sb[:, m, :],
                            op0=Alu.mult, op1=Alu.mult)
                        nc.scalar.activation(out=c_sb[:, m, :],
                                             in_=c_sb[:, m, :], func=AF.Tanh)
                else:
                    for h in range(2):
                        seg = c_sb[:, 2 * h:2 * h + 2, :].rearrange(
                            "p a b -> p (a b)")
                        nc.vector.scalar_tensor_tensor(
                            out=seg,
                            in0=accc[:, 2 * h:2 * h + 2, :].rearrange(
                                "p a b -> p (a b)"),
                            scalar=1.0 / WS,
                            in1=r_sb[:, 2 * h:2 * h + 2, :].rearrange(
                                "p a b -> p (a b)"),
                            op0=Alu.mult, op1=Alu.mult)
                        nc.scalar.activation(out=seg, in_=seg, func=AF.Tanh)
                accu = gate_group(2)
                if g == BLOCKS - 1:
                    for m in range(4):
                        nc.scalar.activation(
                            out=u_sb[:, m, :], in_=accu[:, m, :],
                            func=AF.Sigmoid, scale=1.0 / WS, bias=neg1)
                else:
                    nc.scalar.activation(
                        out=u_sb.rearrange("p a b -> p (a b)"),
                        in_=accu.rearrange("p a b -> p (a b)"),
                        func=AF.Sigmoid, scale=1.0 / WS, bias=neg1)

                if g < BLOCKS - 1:
                    cflat = c_sb.rearrange("p a b -> p (a b)")
                    dunit = dtb_sb[:, 4 * g:4 * g + 4, :].rearrange(
                        "p a b -> p (a b)")
                    uflat = u_sb.rearrange("p a b -> p (a b)")
                    # out = d + u*(c-d), in place in c_sb: sub alternates
                    # Pool/DVE by block parity, mul+add on DVE
                    sub_eng = nc.vector
                    sub_eng.tensor_sub(cflat, cflat, dunit)
                    nc.vector.tensor_mul(cflat, uflat, cflat)
                    nc.vector.tensor_add(cflat, dunit, cflat)
                    nc.sync.dma_start(
                        out=outT[512 * g:512 * (g + 1), :].rearrange(
                            "(s p) b -> p s b", p=P),
                        in_=c_sb)
                else:
                    # last block: per-tile mix so the out DMA overlaps
                    for m in range(4):
                        ct = c_sb[:, m, :]
                        dt_ = dtb_sb[:, 4 * g + m, :]
                        nc.vector.tensor_sub(ct, ct, dt_)
                        nc.vector.tensor_mul(ct, u_sb[:, m, :], ct)
                        nc.vector.tensor_add(ct, dt_, ct)
                        nc.sync.dma_start(
                            out=outT[512 * g + P * m:512 * g + P * (m + 1),
                                     :],
                            in_=ct)

    nc.compile()
    return nc


def _get_program(zb=True):
    if zb not in _PROG:
        _PROG[zb] = _build_program(zb)
    return _PROG[zb]


FP8NP = _ml.float8_e4m3


def _drpack(W, dt):
    """[K, M] -> [K//256, 128, 2, M] DoubleRow-packed, cast to dt."""
    K, M = W.shape
    return np.ascontiguousarray(
        W.reshape(K // 256, 2, P, M).transpose(0, 2, 1, 3)).astype(dt)


def _kpack(W, dt):
    """[K, M] -> [128, K//128, M] (plain k-tiled lhsT), cast to dt."""
    K, M = W.shape
    return np.ascontiguousarray(
        W.reshape(K // P, P, M).transpose(1, 0, 2)).astype(dt)


def _make_const_block(inputs):
    f = lambda a: np.asarray(a, dtype=np.float32)
    cst = np.zeros((P, C_NCOL), dtype=np.float32)
    cst[:, C_BX64:C_BX64 + 16] = WS * np.stack(
        [f(inputs[b]) * f(inputs[g]) for b, g in
         (("b0", "g0"), ("b1", "g1"), ("b2", "g2"), ("b3", "g3"))]
    ).reshape(16, P).T
    cst[:, C_BH064:C_BH064 + 32] = WS * (
        f(inputs["bh0"]) * f(inputs["gh0"])).reshape(32, P).T
    bgt = f(inputs["bg"]).reshape(96, P).T
    cst[:, C_BH164:C_BH164 + 32] = WS * (
        f(inputs["bh1"]) * f(inputs["gh1"])).reshape(32, P).T
    cst[:, C_BG:C_BG + 96] = bgt
    cst[:, C_BG64:C_BG64 + 96] = WS * bgt
    cst[:, C_BGM1:C_BGM1 + 96] = bgt - 1.0
    cst[:, C_EPSH] = WS * WS * EPS
    cst[:, C_NEG1] = -1.0
    return cst


def _prep_inputs(inputs, zb=True):
    """Host-side shard + transpose + quantized weight packing."""
    f = lambda a: np.ascontiguousarray(np.asarray(a), dtype=np.float32)
    stoch = f(inputs["stoch"]).reshape(B, -1)
    deter = f(inputs["deter"])
    action = f(inputs["action"])
    d_emb = f(inputs["d_emb"])

    g0, g1 = f(inputs["g0"]), f(inputs["g1"])
    g2, g3 = f(inputs["g2"]), f(inputs["g3"])
    gh0, gh1 = f(inputs["gh0"]), f(inputs["gh1"])

    W0 = WS * f(inputs["W0"]) * g0
    W1 = WS * f(inputs["W1"]) * g1
    Wh0 = WS * f(inputs["Wh0"]) * gh0.reshape(BLOCKS, 1, OUT_B)
    Wh1 = WS * f(inputs["Wh1"]) * gh1.reshape(BLOCKS, 1, OUT_B)
    Wg = WS * f(inputs["Wg"])

    wh0d = np.ascontiguousarray(np.stack(
        [_drpack(Wh0[g, :OUT_B], FP8NP) for g in range(BLOCKS)]
    ).transpose(0, 2, 1, 3, 4))  # [B, P, 2, 2, M]
    wh1 = np.stack([_kpack(Wh1[g], _ml.bfloat16) for g in range(BLOCKS)])

    bg = f(inputs["bg"])  # [3*DETER], block g segment [1536g:1536(g+1)]
    nkp = 2 if zb else 3
    wgp = np.zeros((BLOCKS, nkp, P, 2, 3 * OUT_B), dtype=FP8NP)
    for g in range(BLOCKS):
        wgp[g, :2] = _drpack(Wg[g], FP8NP)
        if not zb:
            wgp[g, 2, 0, 0, :] = (
                WS * bg[1536 * g:1536 * (g + 1)]).astype(FP8NP)

    cst8 = np.zeros((P, 2 + 2 * BC), dtype=FP8NP)
    cst8[:, 0:2] = 1.0
    cst8[0, 2:2 + BC] = 1.0  # bias-row rhs: partition 0, j=0 ones
    shared = {
        "W0p": _drpack(W0, FP8NP),
        "W1p": np.ascontiguousarray(
            _drpack(W1, FP8NP).transpose(1, 0, 2, 3)),  # [P, 4, 2, M]
        "W2": (WS * f(inputs["W2"]) * g2).astype(np.float32),
        "W3": (WS * f(inputs["W3"]) * g3).astype(np.float32),
        "Wh0x": np.stack([_drpack(Wh0[g, OUT_B:], FP8NP)
                          for g in range(BLOCKS)]),
        "Wh0d": wh0d,
        "Wh1": wh1,
        "Wgp": np.ascontiguousarray(wgp.transpose(0, 2, 1, 3, 4)),
        "cst": _make_const_block(inputs),
        "cst8": cst8,
    }
    # Wh0x packed as [B, pairs, P, 2, M] -> want [B, P, pairs, 2, M]
    shared["Wh0x"] = np.ascontiguousarray(
        shared["Wh0x"].transpose(0, 2, 1, 3, 4))
    # W0p stays [16, P, 2, M] (indexed by pair in the DMA loop)

    in_maps = []
    for c in range(NCORES):
        sl = slice(c * BC, (c + 1) * BC)
        m = dict(shared)
        dT = np.ascontiguousarray(deter[sl].T)
        m["dT8"] = dT.astype(FP8NP)
        m["dTb"] = dT.astype(_ml.bfloat16)
        m["sT8"] = np.ascontiguousarray(stoch[sl].T).astype(FP8NP)
        m["aT"] = np.ascontiguousarray(action[sl].T)
        m["eT"] = np.ascontiguousarray(d_emb[sl].T)
        in_maps.append(m)
    return in_maps


def _gate_bias_zero(inputs):
    return not np.any(np.asarray(inputs["bg"]))


def _run(inputs, trace=False):
    from concourse import bass_utils
    zb = _gate_bias_zero(inputs)
    nc = _get_program(zb)
    in_maps = _prep_inputs(inputs, zb)
    res = bass_utils.run_bass_kernel_spmd(
        nc, in_maps, core_ids=list(range(NCORES)), trace=trace)
    out = np.empty((B, DETER), dtype=np.float32)
    for c in range(NCORES):
        out[c * BC:(c + 1) * BC, :] = \
            np.asarray(res.results[c]["outT"]).astype(np.float32).T
    return out, res.exec_time_ns


def kernel(**inputs):
    out, _ = _run(inputs, trace=False)
    return out


# ---------------------------------------------------------------------------
# benchmarking helper (test-only; the grading path is kernel() above)
# ---------------------------------------------------------------------------

def _bench_generic(nc, in_maps, iters, n_cores=None):
    """Time repeated device executions with device-resident inputs."""
    import time
    import jax
    import concourse.mybir as mybir
    from jax.sharding import Mesh, NamedSharding, PartitionSpec
    from jax.experimental.shard_map import shard_map
    from concourse import bass2jax

    bass2jax.install_neuronx_cc_hook()
    if n_cores is None:
        n_cores = len(in_maps)

    in_names, out_names, out_avals = [], [], []
    for alloc in nc.m.functions[0].allocations:
        if not isinstance(alloc, mybir.MemoryLocationSet):
            continue
        name = alloc.memorylocations[0].name
        pid_name = (nc.partition_id_tensor.name
                    if nc.partition_id_tensor else None)
        if alloc.kind == "ExternalInput":
            if name != pid_name:
                in_names.append(name)
        elif alloc.kind == "ExternalOutput":
            out_names.append(name)
            out_avals.append(jax.core.ShapedArray(
                tuple(alloc.tensor_shape), mybir.dt.np(alloc.dtype)))
    n_params = len(in_names)

    pid_name = nc.partition_id_tensor.name if nc.partition_id_tensor else None
    bind_names = in_names + out_names + ([pid_name] if pid_name else [])

    def _body(*args):
        operands = list(args)
        if pid_name:
            operands.append(bass2jax.partition_id_tensor())
        outs = bass2jax._bass_exec_p.bind(
            *operands,
            out_avals=tuple(out_avals),
            in_names=tuple(bind_names),
            out_names=tuple(out_names),
            lowering_input_output_aliases=(),
            sim_require_finite=True,
            sim_require_nnan=True,
            nc=nc,
        )
        return tuple(outs)

    devices = jax.devices()[:n_cores]
    mesh = Mesh(np.asarray(devices), ("core",))
    nshard = NamedSharding(mesh, PartitionSpec("core"))
    sharded = jax.jit(
        shard_map(_body, mesh=mesh,
                  in_specs=(PartitionSpec("core"),) * (n_params + len(out_names)),
                  out_specs=(PartitionSpec("core"),) * len(out_names),
                  check_rep=False),
        keep_unused=True)

    concat_in = [
        jax.device_put(
            np.concatenate([np.asarray(in_maps[c][nm]) for c in range(n_cores)],
                           axis=0), nshard)
        for nm in in_names]
    concat_zeros = [
        jax.device_put(
            np.zeros((n_cores * a.shape[0], *a.shape[1:]), a.dtype), nshard)
        for a in out_avals]

    outs = sharded(*concat_in, *concat_zeros)
    jax.block_until_ready(outs)

    BATCH = 6
    diffs = []
    for _ in range(iters):
        t0 = time.perf_counter()
        outs = sharded(*concat_in, *concat_zeros)
        jax.block_until_ready(outs)
        t1 = time.perf_counter()
        for _ in range(BATCH):
            outs = sharded(*concat_in, *concat_zeros)
        jax.block_until_ready(outs)
        t2 = time.perf_counter()
        diffs.append((t2 - t1) - (t1 - t0))
    diffs.sort()
    per_iter_ns = diffs[len(diffs) // 2] / (BATCH - 1) * 1e9
    return outs, per_iter_ns


_TINY = None


def _tiny_program():
    """Near-noop program with the SAME input/output signature, to measure
    axon dispatch overhead differentially."""
    global _TINY
    if _TINY is None:
        nc = bacc.Bacc(trn_type="TRN2", target_bir_lowering=False, debug=False)
        d = {"dT8": ((DETER, BC), fp8), "dTb": ((DETER, BC), bf16),
             "sT8": ((STOCH, BC), fp8), "aT": ((ACT_DIM, BC), f32),
             "eT": ((DEMB, BC), f32), "W0p": ((16, P, 2, HIDDEN), fp8),
             "W1p": ((P, 4, 2, HIDDEN), fp8), "W2": ((ACT_DIM, HIDDEN), f32),
             "W3": ((DEMB, HIDDEN), f32),
             "Wh0x": ((BLOCKS, P, 8, 2, OUT_B), fp8),
             "Wh0d": ((BLOCKS, P, 2, 2, OUT_B), fp8),
             "Wh1": ((BLOCKS, P, 4, OUT_B), bf16),
             "Wgp": ((BLOCKS, P, 2, 2, 3 * OUT_B), fp8),
             "cst": ((P, C_NCOL), f32), "cst8": ((P, 2 + 2 * BC), fp8)}
        aps = {k: nc.dram_tensor(k, list(s), dt, kind="ExternalInput").ap()
               for k, (s, dt) in d.items()}
        outT = nc.dram_tensor("outT", [DETER, BC], bf16,
                              kind="ExternalOutput").ap()
        with tile.TileContext(nc) as tc:
            with tc.tile_pool(name="t", bufs=2) as pool:
                t = pool.tile([P, 4, BC], bf16)
                nc.sync.dma_start(
                    out=t, in_=aps["dTb"][:512, :].rearrange(
                        "(s p) b -> p s b", p=P))
                for g in range(BLOCKS):
                    nc.sync.dma_start(
                        out=outT[512 * g:512 * (g + 1), :].rearrange(
                            "(s p) b -> p s b", p=P),
                        in_=t)
        nc.compile()
        _TINY = nc
    return _TINY


def _bench_overhead(inputs, iters=20):
    nc = _tiny_program()
    in_maps = _prep_inputs(inputs, True)
    _, t = _bench_generic(nc, in_maps, iters)
    return t


def _bench(inputs, iters=20):
    zb = _gate_bias_zero(inputs)
    nc = _get_program(zb)
    in_maps = _prep_inputs(inputs, zb)
    outs, per_iter_ns = _bench_generic(nc, in_maps, iters)
    res = np.asarray(outs[0]).reshape(NCORES, DETER, BC)
    out = np.empty((B, DETER), dtype=np.float32)
    for c in range(NCORES):
        out[c * BC:(c + 1) * BC, :] = res[c].astype(np.float32).T
    return out, per_iter_ns


# revision 41
# speedup vs baseline: 1.0002x; 1.0002x over previous
"""Trainium2 Bass kernel for the Deter GRU-MLP block (RSSM deter update).

Sharding: data-parallel over batch B=4096 across 8 NeuronCores (512 rows
each), all parameters replicated; no collectives.

Design (mixed fp8/bf16, measured 215us TimelineSim vs 407us baseline,
hardware-validated rel-max err 1.42e-2):
- Activations live transposed in SBUF (features on partitions, batch on
  the 512-wide free axis); weights consumed in natural [K, M] layout.
- Big GEMMs run as fp8(e4m3) DoubleRow matmuls (0.5 cyc/row, two k-tiles
  per instruction, host-packed [pairs, 128, 2, M] weights): branch W0/W1,
  both slices of the block-diagonal L0, and the GRU gate GEMM.  L1 stays
  bf16 for accuracy (numpy quantization model: fp8_sim3.py).
- All weights are scaled by 64 on the host (fp8 normal range) with norm
  gains folded in, so pre-norm PSUM values are 64*y.  The rmsnorm absorbs
  the scale: squares are (64y)^2 in bf16, the sqrt fold yields
  1/(64*rms) directly, and the gate nonlinearities use the ACT scale
  operand (sigmoid(acc/64 + b)).
- Per 4-tile norm unit: PSUM->SBUF copies ride the otherwise-idle ACT
  engine (Identity+bias) in L0/L1, squares are a 2x-rate bf16 DVE
  multiply, the sum-of-squares reduction is a ones-vector matmul, and
  silu is a batched ACT sigmoid + DVE multiply casting to fp8/bf16 on
  write.  The four branch norms share two fused sqrt/recip/broadcasts.
- Engines are in-order, so loops are software-pipelined: block g+1's
  normalize+silu is emitted during block g's GEMMs, each block's ss
  matmuls are deferred past the next block's GEMMs, and the first/last
  blocks around each norm barrier run per-tile (or per-half for the gate
  sigmoid/stt/tanh chain) to shorten restart and drain latency.
  Cross-phase weight prefetch tiles live in enclosing pool scopes so
  their DMAs do not inherit false WARs from reused SBUF regions.
- deter is read once as fp8 (branch-0 GEMM + L0 dg slice, resident) and
  once as bf16 (GRU mix operand, streamed during the gates phase);
  output is written bf16 and upcast on the host.
- Hardware-legality notes baked in: gpsimd(Pool) cannot touch PSUM and
  cannot run TensorScalarPtr; DoubleRow Ldweights rejects degenerate
  single-column stationary tiles; Sqrt lives in a different ACT table
  set than Sigmoid/Tanh, so table switches are pre-triggered by tiny
  warm-up ops off the critical path.
"""

import os
import sys
from contextlib import ExitStack

import numpy as np
import ml_dtypes as _ml

for _p in ("/opt/trn_rl_repo", "/opt/pypackages"):
    if os.path.isdir(_p) and _p not in sys.path:
        sys.path.insert(0, _p)

os.environ.setdefault("MYCRO_LOCAL_CACHE", "1")

import concourse.bass as bass  # noqa: E402
import concourse.bacc as bacc  # noqa: E402
import concourse.mybir as mybir  # noqa: E402
import concourse.tile as tile  # noqa: E402

# ---- problem constants (hardcoded; kernel.py must be self-contained) ----
P = 128
B = 4096
NCORES = 8
BC = B // NCORES  # 512 batch columns per core
DETER = 4096
STOCH = 1024
ACT_DIM = 32
DEMB = 16
HIDDEN = 512
BLOCKS = 8
OUT_B = DETER // BLOCKS  # 512
IN_B0 = 4 * HIDDEN + OUT_B  # 2560
EPS = 1e-4

ND = DETER // P       # 32 deter k/n tiles
NX = 4 * HIDDEN // P  # 16 x k tiles
WS = 64.0             # weight scale folded into rmsnorm / gate scales

# const-block column layout (single [P, C_NCOL] f32 DRAM input)
C_BX64 = 0            # 16: 64*(branch bias * gain)
C_BH064 = 16          # 32: 64*(bh0 * gh0)
C_BH164 = 48          # 32: 64*(bh1 * gh1)
C_BG = 80             # 96: bg (unscaled, sigmoid bias)
C_BG64 = 176          # 96: 64*bg (cand stt bias)
C_BGM1 = 272          # 96: bg - 1 (update sigmoid bias)
C_EPSH = 368          # 1: 4096*EPS
C_NEG1 = 369          # 1: -1.0 (update-gate bias)
C_NCOL = 370

f32 = mybir.dt.float32
f32r = mybir.dt.float32r
bf16 = mybir.dt.bfloat16
fp8 = mybir.dt.float8e4

# (L1 is the only bf16 GEMM; see fp8_sim3.py for the quantization study)

_PROG = {}


def _r(ap):
    return ap.bitcast(f32r)


def _build_program(zb):
    """Build the single-core SPMD Bass program (same on all 8 cores).

    zb: gate biases are all zero -> skip the bias K-row in the gate GEMM
    (saves a weight pair per m-tile and 3MB of DMA per core)."""
    AF = mybir.ActivationFunctionType
    Alu = mybir.AluOpType
    DR = mybir.MatmulPerfMode.DoubleRow
    nc = bacc.Bacc(trn_type="TRN2", target_bir_lowering=False, debug=False)

    def din(name, shape, dt=f32):
        return nc.dram_tensor(name, list(shape), dt, kind="ExternalInput").ap()

    dT8 = din("dT8", (DETER, BC), fp8)
    dTb = din("dTb", (DETER, BC), bf16)
    sT8 = din("sT8", (STOCH, BC), fp8)
    aT = din("aT", (ACT_DIM, BC))
    eT = din("eT", (DEMB, BC))
    W0p = din("W0p", (16, P, 2, HIDDEN), fp8)
    W1p = din("W1p", (P, 4, 2, HIDDEN), fp8)
    W2 = din("W2", (ACT_DIM, HIDDEN))
    W3 = din("W3", (DEMB, HIDDEN))
    Wh0x = din("Wh0x", (BLOCKS, P, 8, 2, OUT_B), fp8)
    Wh0d = din("Wh0d", (BLOCKS, P, 2, 2, OUT_B), fp8)
    Wh1 = din("Wh1", (BLOCKS, P, 4, OUT_B), bf16)
    NKP = 2 if zb else 3
    Wgp = din("Wgp", (BLOCKS, P, NKP, 2, 3 * OUT_B), fp8)
    cst = din("cst", (P, C_NCOL))
    cst8 = din("cst8", (P, 2 + 2 * BC), fp8)
    outT = nc.dram_tensor("outT", [DETER, BC], bf16,
                          kind="ExternalOutput").ap()

    with tile.TileContext(nc) as tc, ExitStack() as top:
        consts = top.enter_context(tc.tile_pool(name="consts", bufs=1))
        cst_sb = consts.tile([P, C_NCOL], f32)
        nc.sync.dma_start(out=_r(cst_sb), in_=_r(cst))
        cst8_sb = consts.tile([P, 2 + 2 * BC], fp8)
        nc.sync.dma_start(out=cst8_sb, in_=cst8)
        ones8 = cst8_sb[:, 0:2]
        ones_b16 = consts.tile([P, 1], bf16)
        nc.vector.memset(ones_b16, 1.0)
        onesp = cst8_sb[:, 2:2 + 2 * BC].rearrange("p (j b) -> p j b", j=2)
        bgm1 = cst_sb[:, C_BGM1:C_BGM1 + 96]
        epsh = cst_sb[:1, C_EPSH:C_EPSH + 1]
        neg1 = cst_sb[:, C_NEG1:C_NEG1 + 1]

        # resident regions
        mainp = top.enter_context(tc.tile_pool(name="mainp", bufs=1))
        main_sb = mainp.tile([P, ND, BC], bf16)   # pre-norm y64 / h0n
        dtbp = top.enter_context(tc.tile_pool(name="dtbp", bufs=1))
        dtb_sb = dtbp.tile([P, ND, BC], bf16)     # deter bf16 (L0 dg + mix)
        x8p = top.enter_context(tc.tile_pool(name="x8p", bufs=1))
        x8_sb = x8p.tile([P, NX, BC], fp8)        # branch outputs (L0 rhs)
        h1p = top.enter_context(tc.tile_pool(name="h1p", bufs=1))
        h1n8 = h1p.tile([P, ND, BC], fp8)         # L1 normalized (gates rhs)

        ysqp = top.enter_context(tc.tile_pool(name="ysqp", bufs=3))
        invp = top.enter_context(tc.tile_pool(name="invp", bufs=2))
        invbp = top.enter_context(tc.tile_pool(name="invbp", bufs=2))
        sigp = top.enter_context(tc.tile_pool(name="sigp", bufs=3))

        def act_warm(func, name):
            """Trigger an ACT table switch off the critical path."""
            t = invp.tile([1, 1], f32, tag="warm", name=f"warm_{name}")
            nc.scalar.activation(out=t, in_=epsh, func=func)

        def finish_norm(ss_flat, D, width, name):
            """invb = 1/(64*sqrt(ss/D + eps)), bf16, broadcast to all
            partitions.  ss_flat: [1, width*BC] (PSUM)."""
            sq = invp.tile([1, width * BC], f32, tag="sq", name=f"sq_{name}")
            nc.scalar.activation(out=sq, in_=ss_flat, func=AF.Sqrt,
                                 bias=epsh, scale=1.0 / D)
            act_warm(AF.Sigmoid, f"sg_{name}")  # reload hidden under recip
            inv1 = invp.tile([1, width * BC], bf16, tag="inv1",
                             name=f"inv1_{name}")
            with nc.allow_low_precision(reason="bf16 rstd broadcast"):
                nc.vector.reciprocal(inv1, sq)
            invb = invbp.tile([P, width * BC], bf16, tag="invb",
                              name=f"invb_{name}")
            nc.gpsimd.partition_broadcast(invb, inv1)
            return invb

        def norm_silu_unit(unit_y, invb, out_unit, name, per_tile=False,
                           mul_pool=False):
            """out_unit <- silu(unit_y * invb) = t*sigmoid(t), t=y*inv.
            per_tile=True pipelines at tile granularity (lower latency
            right after a norm barrier)."""
            mul_eng = nc.gpsimd if mul_pool else nc.vector
            for m in range(4):
                nc.vector.tensor_mul(unit_y[:, m, :], unit_y[:, m, :],
                                     invb[:, m * BC:(m + 1) * BC]
                                     if invb.shape[-1] == 4 * BC else invb)
            s = sigp.tile([P, 4, BC], bf16, tag="sig", name=f"sig_{name}")
            if per_tile:
                for m in range(4):
                    nc.scalar.activation(out=s[:, m, :], in_=unit_y[:, m, :],
                                         func=AF.Sigmoid)
                    mul_eng.tensor_mul(out_unit[:, m, :], unit_y[:, m, :],
                                       s[:, m, :])
            else:
                nc.scalar.activation(
                    out=s.rearrange("p a b -> p (a b)"),
                    in_=unit_y.rearrange("p a b -> p (a b)"), func=AF.Sigmoid)
                mul_eng.tensor_mul(
                    out_unit.rearrange("p a b -> p (a b)"),
                    unit_y.rearrange("p a b -> p (a b)"),
                    s.rearrange("p a b -> p (a b)"))

        # ============ phases A, L0, L1 (shared PSUM layout) ============
        with ExitStack() as ph_al:
            psum_acc = ph_al.enter_context(
                tc.tile_pool(name="pacc", bufs=5, space="PSUM"))
            psum_ss = ph_al.enter_context(
                tc.tile_pool(name="pss", bufs=1, space="PSUM"))
            d8pool = ph_al.enter_context(tc.tile_pool(name="d8r", bufs=1))
            d8sb = d8pool.tile([P, ND, BC], fp8)  # fp8 deter (br0 + L0 dg)

            def unit_post(unit_y, accs, b64col0, ss, ss_first, ss_last,
                          name, act_copy, sq_pool=False, presum=False):
                """copy accs (+64*bias) into unit_y (bf16) -- on ACT
                (Identity) when act_copy else DVE; squares (64y)^2 bf16.
                Returns a closure emitting the 4 ss ones-matmuls, so the
                caller can defer them past the next block's GEMMs (the PE
                is in-order; immediate ss would stall it on this block's
                elementwise post)."""
                for m in range(4):
                    bcol = cst_sb[:, b64col0 + m:b64col0 + m + 1]
                    if act_copy and (act_copy == 2 or m % 2 == 0):
                        nc.scalar.activation(out=unit_y[:, m, :],
                                             in_=accs[m], func=AF.Identity,
                                             bias=bcol)
                    else:
                        nc.vector.tensor_scalar_add(unit_y[:, m, :],
                                                    accs[m], bcol)
                ysq = ysqp.tile([P, 4, BC], bf16, tag="ysq",
                                name=f"ysq_{name}")
                if sq_pool:
                    for half in range(2):
                        seg = unit_y[:, 2 * half:2 * half + 2, :].rearrange(
                            "p a b -> p (a b)")
                        nc.gpsimd.tensor_mul(
                            ysq[:, 2 * half:2 * half + 2, :].rearrange(
                                "p a b -> p (a b)"), seg, seg)
                else:
                    for t in range(4):
                        nc.vector.tensor_mul(ysq[:, t, :], unit_y[:, t, :],
                                             unit_y[:, t, :])

                if presum:
                    # fold 4 tiles into 1 on the (idle) DVE; the PE then
                    # does a single ones-matmul instead of four
                    nc.vector.tensor_add(ysq[:, 0, :], ysq[:, 0, :],
                                         ysq[:, 1, :])
                    nc.vector.tensor_add(ysq[:, 2, :], ysq[:, 2, :],
                                         ysq[:, 3, :])
                    nc.vector.tensor_add(ysq[:, 0, :], ysq[:, 0, :],
                                         ysq[:, 2, :])

                    def emit_ss():
                        nc.tensor.matmul(ss, lhsT=ones_b16, rhs=ysq[:, 0, :],
                                         start=ss_first, stop=ss_last)
                else:
                    def emit_ss():
                        for t in range(4):
                            nc.tensor.matmul(
                                ss, lhsT=ones_b16, rhs=ysq[:, t, :],
                                start=(ss_first and t == 0),
                                stop=(ss_last and t == 3))
                return emit_ss

            # ---------------- phase A: four input branches ----------------
            with ExitStack() as ph_a:
                w0p_pool = ph_a.enter_context(
                    tc.tile_pool(name="w0p", bufs=8))
                sp = ph_a.enter_context(tc.tile_pool(name="sp", bufs=1))

                sT_sb = sp.tile([P, STOCH // P, BC], fp8)
                aT_sb = sp.tile([ACT_DIM, BC], f32)
                eT_sb = sp.tile([DEMB, BC], f32)
                an_sb = sp.tile([ACT_DIM, BC], f32)
                w3t = sp.tile([DEMB, HIDDEN], f32)
                w2t = sp.tile([ACT_DIM, HIDDEN], f32)
                w1t = sp.tile([P, 4, 2, HIDDEN], fp8)

                # the 4MB branch-0 stream is the phase-A long pole: first
                w0_slabs = []
                for c in range(8):
                    nc.sync.dma_start(
                        out=d8sb[:, 4 * c:4 * c + 4, :],
                        in_=dT8[512 * c:512 * (c + 1), :].rearrange(
                            "(s p) b -> p s b", p=P))
                    w0 = w0p_pool.tile([P, 2, 2, HIDDEN], fp8, tag="w0",
                                       name=f"w0_{c}")
                    nc.sync.dma_start(
                        out=w0, in_=W0p[2 * c:2 * c + 2].rearrange(
                            "s p j m -> p s j m"))
                    w0_slabs.append(w0)
                    if c == 0:
                        nc.sync.dma_start(out=_r(eT_sb), in_=_r(eT))
                        nc.sync.dma_start(out=_r(w3t), in_=_r(W3))
                        nc.sync.dma_start(out=aT_sb, in_=aT)
                        nc.sync.dma_start(out=_r(w2t), in_=_r(W2))
                    if c == 1:
                        nc.sync.dma_start(
                            out=sT_sb,
                            in_=sT8.rearrange("(s p) b -> p s b", p=P))
                        nc.sync.dma_start(out=w1t, in_=W1p)

                # action preprocess: a / max(|a|, 1)
                ab = sp.tile([ACT_DIM, BC], f32)
                nc.scalar.activation(out=ab, in_=aT_sb, func=AF.Abs)
                act_warm(AF.Sqrt, "br")
                nc.vector.tensor_scalar_max(ab, ab, 1.0)
                nc.vector.reciprocal(ab, ab)
                nc.vector.tensor_mul(_r(an_sb), aT_sb, ab)

                def branch_small(br, wt, rhs):
                    accs = []
                    for m in range(4):
                        acc = psum_acc.tile([P, BC], f32, tag="acc",
                                            name=f"acc_br{br}_{m}")
                        nc.tensor.matmul(acc,
                                         lhsT=_r(wt[:, m * P:(m + 1) * P]),
                                         rhs=_r(rhs), start=True, stop=True)
                        accs.append(acc)
                    return accs

                def branch_dr(br, npairs, wslab, rhs_pair):
                    accs = [psum_acc.tile([P, BC], f32, tag="acc",
                                          name=f"acc_br{br}_{m}")
                            for m in range(4)]
                    for kp in range(npairs):
                        w = wslab(kp)
                        rhs = rhs_pair(kp)
                        for m in range(4):
                            nc.tensor.matmul(
                                accs[m], lhsT=w[:, :, m * P:(m + 1) * P],
                                rhs=rhs, start=(kp == 0),
                                stop=(kp == npairs - 1), perf_mode=DR)
                    return accs

                # branches 1-3 share one fused norm (single sqrt/recip/
                # broadcast over 3 stacked ss slots); branch 0 -- whose GEMM
                # is gated on the big dT8/W0p DMA stream -- normalizes on
                # its own so the other three silus overlap that stream.
                ss3 = psum_ss.tile([1, 2, BC], f32, tag="ss3", name="ss3")

                def branch_post(br, accs, ss):
                    unit = main_sb[:, 4 * br:4 * br + 4, :]
                    unit_post(unit, accs, C_BX64 + 4 * br, ss,
                              True, True, f"br{br}", act_copy=2,
                              sq_pool=(br == 3))()

                branch_post(3, branch_small(3, w3t, eT_sb), ss3[:, 0, :])
                branch_post(2, branch_small(2, w2t, an_sb), ss3[:, 1, :])
                ssb1 = psum_ss.tile([1, BC], f32, tag="ss", name="ssb1")
                branch_post(1, branch_dr(
                    1, 4, lambda kp: w1t[:, kp, :, :],
                    lambda kp: sT_sb[:, 2 * kp:2 * kp + 2, :]), ssb1)
                invb3 = finish_norm(ss3.rearrange("o a b -> o (a b)"),
                                    HIDDEN, 2, "br32")
                for i, br in enumerate((3, 2)):
                    norm_silu_unit(
                        main_sb[:, 4 * br:4 * br + 4, :],
                        invb3[:, i * BC:(i + 1) * BC],
                        x8_sb[:, 4 * br:4 * br + 4, :], f"br{br}",
                        mul_pool=(br == 3))
                invb1b = finish_norm(ssb1, HIDDEN, 1, "br1")
                norm_silu_unit(main_sb[:, 4:8, :], invb1b,
                               x8_sb[:, 4:8, :], "br1")

                ssb0 = psum_ss.tile([1, BC], f32, tag="ss", name="ssb0")
                branch_post(0, branch_dr(
                    0, 16, lambda kp: w0_slabs[kp // 2][:, kp % 2, :, :],
                    lambda kp: d8sb[:, 2 * kp:2 * kp + 2, :]), ssb0)
                invb0b = finish_norm(ssb0, HIDDEN, 1, "br0")
                norm_silu_unit(main_sb[:, 0:4, :], invb0b,
                               x8_sb[:, 0:4, :], "br0", per_tile=True)

            # ---------- L0: BlockLinear(2560 -> 512/block) ----------
            with ExitStack() as ph_l:
                wh0xp = ph_l.enter_context(
                    tc.tile_pool(name="wh0xp", bufs=2))
                wh0dp = ph_l.enter_context(
                    tc.tile_pool(name="wh0dp", bufs=2))
                wh1p = ph_l.enter_context(tc.tile_pool(name="wh1p", bufs=3))

                ss0 = psum_ss.tile([1, BC], f32, tag="ss", name="ss_l0")
                pend0 = []
                for g in range(BLOCKS):
                    wd = wh0dp.tile([P, 2, 2, OUT_B], fp8, tag="wh0d",
                                    name=f"wh0d_{g}")
                    nc.sync.dma_start(out=wd, in_=Wh0d[g])
                    wx = wh0xp.tile([P, 8, 2, OUT_B], fp8, tag="wh0x",
                                    name=f"wh0x_{g}")
                    nc.sync.dma_start(out=wx, in_=Wh0x[g])
                    accs = [psum_acc.tile([P, BC], f32, tag="acc",
                                          name=f"acc_h0_{g}_{m}")
                            for m in range(4)]
                    # dg (fp8 DR) matmuls for all m first: they only need
                    # the resident fp8 deter + wd, so the PE can run them
                    # while the branch silu (x8) is still finishing
                    for m in range(4):
                        for kp in range(2):
                            nc.tensor.matmul(
                                accs[m],
                                lhsT=wd[:, kp, :, m * P:(m + 1) * P],
                                rhs=d8sb[:, 4 * g + 2 * kp:4 * g + 2 * kp + 2,
                                         :],
                                start=(kp == 0), stop=False, perf_mode=DR)
                    for m in range(4):
                        for kp in (2, 3, 4, 5, 6, 7, 0, 1):
                            nc.tensor.matmul(
                                accs[m], lhsT=wx[:, kp, :, m * P:(m + 1) * P],
                                rhs=x8_sb[:, 2 * kp:2 * kp + 2, :],
                                start=False, stop=(kp == 1), perf_mode=DR)
                    # deferred ss for the previous block: keeps the PE from
                    # stalling on this block's elementwise post
                    if pend0:
                        pend0.pop()()
                    pend0.append(unit_post(
                        main_sb[:, 4 * g:4 * g + 4, :], accs,
                        C_BH064 + 4 * g, ss0, g == 0, g == BLOCKS - 1,
                        f"l0_{g}", act_copy=True, sq_pool=(g % 2 == 0)))
                    if g == 5:
                        act_warm(AF.Sqrt, "l0")
                pend0.pop()()
                invb0 = finish_norm(ss0, DETER, 1, "l0")

                # --------- L1 (bf16), interleaved with the L0 norm ---------
                ss1 = psum_ss.tile([1, BC], f32, tag="ss", name="ss_l1")
                # h0n = silu(norm(h0)) in place; block 0 primed per-tile,
                # block g+1's silu is emitted during block g (in-order
                # engines would otherwise serialize consecutive blocks)
                norm_silu_unit(main_sb[:, 0:4, :], invb0,
                               main_sb[:, 0:4, :], "l1_0", per_tile=True)
                pend1 = []
                for g in range(BLOCKS):
                    unit = main_sb[:, 4 * g:4 * g + 4, :]
                    # stream the bf16 deter (GRU mix operand) here, where
                    # the DMA engines are otherwise idle
                    nc.sync.dma_start(
                        out=dtb_sb[:, 4 * g:4 * g + 4, :],
                        in_=dTb[512 * g:512 * (g + 1), :].rearrange(
                            "(s p) b -> p s b", p=P))
                    w1h = wh1p.tile([P, 4, OUT_B], bf16, tag="wh1",
                                    name=f"wh1_{g}")
                    nc.sync.dma_start(out=w1h, in_=Wh1[g])
                    accs = [psum_acc.tile([P, BC], f32, tag="acc",
                                          name=f"acc_h1_{g}_{m}")
                            for m in range(4)]
                    for m in range(4):
                        for kk in range(4):
                            nc.tensor.matmul(
                                accs[m], lhsT=w1h[:, kk, m * P:(m + 1) * P],
                                rhs=unit[:, kk, :],
                                start=(kk == 0), stop=(kk == 3))
                    if g + 1 < BLOCKS:
                        nxt = main_sb[:, 4 * (g + 1):4 * (g + 1) + 4, :]
                        norm_silu_unit(nxt, invb0, nxt, f"l1_{g + 1}")
                    if pend1:
                        pend1.pop()()
                    pend1.append(unit_post(
                        unit, accs, C_BH164 + 4 * g, ss1,
                        g == 0, g == BLOCKS - 1, f"l1_{g}",
                        act_copy=True, sq_pool=(g % 2 == 0)))
                    if g == 5:
                        act_warm(AF.Sqrt, "l1")
                pend1.pop()()
                invb1 = finish_norm(ss1, DETER, 1, "l1")

        # ------------- GRU gates + final mix (per block) -------------
        with ExitStack() as ph_g:
            gpsum = ph_g.enter_context(
                tc.tile_pool(name="gpsum", bufs=2, space="PSUM"))
            wgp = ph_g.enter_context(tc.tile_pool(name="wgpool", bufs=3))
            grup = ph_g.enter_context(tc.tile_pool(name="grup", bufs=3))

            norm_silu_unit(main_sb[:, 0:4, :], invb1, h1n8[:, 0:4, :],
                           "g0", per_tile=True)
            for g in range(BLOCKS):
                wg = wgp.tile([P, NKP, 2, 3 * OUT_B], fp8, tag="wg",
                              name=f"wg_{g}")
                nc.sync.dma_start(out=wg, in_=Wgp[g])
                r_sb = grup.tile([P, 4, BC], bf16, tag="r", name=f"r_{g}")
                c_sb = grup.tile([P, 4, BC], bf16, tag="c", name=f"c_{g}")
                u_sb = grup.tile([P, 4, BC], bf16, tag="u", name=f"u_{g}")

                def gate_group(grp):
                    """12 DoubleRow matmuls (2 data pairs + bias pair) into
                    a 4-bank PSUM group for gate third grp."""
                    acc4 = gpsum.tile([P, 4, BC], f32, tag="g4",
                                      name=f"acc_g{g}_{grp}")
                    for m in range(4):
                        mm = 4 * grp + m
                        for kp in range(NKP):
                            rhs = (onesp if kp == 2 else
                                   h1n8[:, 4 * g + 2 * kp:4 * g + 2 * kp + 2, :])
                            nc.tensor.matmul(
                                acc4[:, m, :],
                                lhsT=wg[:, kp, :, mm * P:(mm + 1) * P],
                                rhs=rhs, start=(kp == 0),
                                stop=(kp == NKP - 1), perf_mode=DR)
                    return acc4

                accr = gate_group(0)
                if g == BLOCKS - 1:
                    for m in range(4):
                        nc.scalar.activation(
                            out=r_sb[:, m, :], in_=accr[:, m, :],
                            func=AF.Sigmoid, scale=1.0 / WS)
                else:
                    for h in range(2):
                        nc.scalar.activation(
                            out=r_sb[:, 2 * h:2 * h + 2, :].rearrange(
                                "p a b -> p (a b)"),
                            in_=accr[:, 2 * h:2 * h + 2, :].rearrange(
                                "p a b -> p (a b)"),
                            func=AF.Sigmoid, scale=1.0 / WS)
                if g + 1 < BLOCKS:
                    nxt = main_sb[:, 4 * (g + 1):4 * (g + 1) + 4, :]
                    norm_silu_unit(nxt, invb1,
                                   h1n8[:, 4 * (g + 1):4 * (g + 1) + 4, :],
                                   f"g{g + 1}", mul_pool=False)
                accc = gate_group(1)
                if g == BLOCKS - 1:
                    for m in range(4):
                        nc.vector.scalar_tensor_tensor(
                            out=c_sb[:, m, :], in0=accc[:, m, :],
                            scalar=1.0 / WS, in1=r_sb[:, m, :],
                            op0=Alu.mult, op1=Alu.mult)
                        nc.scalar.activation(out=c_sb[:, m, :],
                                             in_=c_sb[:, m, :], func=AF.Tanh)
                else:
                    for h in range(2):
                        seg = c_sb[:, 2 * h:2 * h + 2, :].rearrange(
                            "p a b -> p (a b)")
                        nc.vector.scalar_tensor_tensor(
                            out=seg,
                            in0=accc[:, 2 * h:2 * h + 2, :].rearrange(
                                "p a b -> p (a b)"),
                            scalar=1.0 / WS,
                            in1=r_sb[:, 2 * h:2 * h + 2, :].rearrange(
                                "p a b -> p (a b)"),
                            op0=Alu.mult, op1=Alu.mult)
                        nc.scalar.activation(out=seg, in_=seg, func=AF.Tanh)
                accu = gate_group(2)
                if g == BLOCKS - 1:
                    for m in range(4):
                        nc.scalar.activation(
                            out=u_sb[:, m, :], in_=accu[:, m, :],
                            func=AF.Sigmoid, scale=1.0 / WS, bias=neg1)
                else:
                    nc.scalar.activation(
                        out=u_sb.rearrange("p a b -> p (a b)"),
                        in_=accu.rearrange("p a b -> p (a b)"),
                        func=AF.Sigmoid, scale=1.0 / WS, bias=neg1)

                if g < BLOCKS - 1:
                    cflat = c_sb.rearrange("p a b -> p (a b)")
                    dunit = dtb_sb[:, 4 * g:4 * g + 4, :].rearrange(
                        "p a b -> p (a b)")
                    uflat = u_sb.rearrange("p a b -> p (a b)")
                    # out = d + u*(c-d), in place in c_sb: sub alternates
                    # Pool/DVE by block parity, mul+add on DVE
                    sub_eng = nc.vector
                    sub_eng.tensor_sub(cflat, cflat, dunit)
                    nc.vector.tensor_mul(cflat, uflat, cflat)
                    nc.vector.tensor_add(cflat, dunit, cflat)
                    nc.sync.dma_start(
                        out=outT[512 * g:512 * (g + 1), :].rearrange(
                            "(s p) b -> p s b", p=P),
                        in_=c_sb)
                else:
                    # last block: per-tile mix so the out DMA overlaps
                    for m in range(4):
                        ct = c_sb[:, m, :]
                        dt_ = dtb_sb[:, 4 * g + m, :]
                        nc.vector.tensor_sub(ct, ct, dt_)
                        nc.vector.tensor_mul(ct, u_sb[:, m, :], ct)
                        nc.vector.tensor_add(ct, dt_, ct)
                        nc.sync.dma_start(
                            out=outT[512 * g + P * m:512 * g + P * (m + 1),
                                     :],
                            in_=ct)

    nc.compile()
    return nc


def _get_program(zb=True):
    if zb not in _PROG:
        _PROG[zb] = _build_program(zb)
    return _PROG[zb]


FP8NP = _ml.float8_e4m3


def _drpack(W, dt):
    """[K, M] -> [K//256, 128, 2, M] DoubleRow-packed, cast to dt."""
    K, M = W.shape
    return np.ascontiguousarray(
        W.reshape(K // 256, 2, P, M).transpose(0, 2, 1, 3)).astype(dt)


def _kpack(W, dt):
    """[K, M] -> [128, K//128, M] (plain k-tiled lhsT), cast to dt."""
    K, M = W.shape
    return np.ascontiguousarray(
        W.reshape(K // P, P, M).transpose(1, 0, 2)).astype(dt)


def _make_const_block(inputs):
    f = lambda a: np.asarray(a, dtype=np.float32)
    cst = np.zeros((P, C_NCOL), dtype=np.float32)
    cst[:, C_BX64:C_BX64 + 16] = WS * np.stack(
        [f(inputs[b]) * f(inputs[g]) for b, g in
         (("b0", "g0"), ("b1", "g1"), ("b2", "g2"), ("b3", "g3"))]
    ).reshape(16, P).T
    cst[:, C_BH064:C_BH064 + 32] = WS * (
        f(inputs["bh0"]) * f(inputs["gh0"])).reshape(32, P).T
    bgt = f(inputs["bg"]).reshape(96, P).T
    cst[:, C_BH164:C_BH164 + 32] = WS * (
        f(inputs["bh1"]) * f(inputs["gh1"])).reshape(32, P).T
    cst[:, C_BG:C_BG + 96] = bgt
    cst[:, C_BG64:C_BG64 + 96] = WS * bgt
    cst[:, C_BGM1:C_BGM1 + 96] = bgt - 1.0
    cst[:, C_EPSH] = WS * WS * EPS
    cst[:, C_NEG1] = -1.0
    return cst


def _prep_inputs(inputs, zb=True):
    """Host-side shard + transpose + quantized weight packing."""
    f = lambda a: np.ascontiguousarray(np.asarray(a), dtype=np.float32)
    stoch = f(inputs["stoch"]).reshape(B, -1)
    deter = f(inputs["deter"])
    action = f(inputs["action"])
    d_emb = f(inputs["d_emb"])

    g0, g1 = f(inputs["g0"]), f(inputs["g1"])
    g2, g3 = f(inputs["g2"]), f(inputs["g3"])
    gh0, gh1 = f(inputs["gh0"]), f(inputs["gh1"])

    W0 = WS * f(inputs["W0"]) * g0
    W1 = WS * f(inputs["W1"]) * g1
    Wh0 = WS * f(inputs["Wh0"]) * gh0.reshape(BLOCKS, 1, OUT_B)
    Wh1 = WS * f(inputs["Wh1"]) * gh1.reshape(BLOCKS, 1, OUT_B)
    Wg = WS * f(inputs["Wg"])

    wh0d = np.ascontiguousarray(np.stack(
        [_drpack(Wh0[g, :OUT_B], FP8NP) for g in range(BLOCKS)]
    ).transpose(0, 2, 1, 3, 4))  # [B, P, 2, 2, M]
    wh1 = np.stack([_kpack(Wh1[g], _ml.bfloat16) for g in range(BLOCKS)])

    bg = f(inputs["bg"])  # [3*DETER], block g segment [1536g:1536(g+1)]
    nkp = 2 if zb else 3
    wgp = np.zeros((BLOCKS, nkp, P, 2, 3 * OUT_B), dtype=FP8NP)
    for g in range(BLOCKS):
        wgp[g, :2] = _drpack(Wg[g], FP8NP)
        if not zb:
            wgp[g, 2, 0, 0, :] = (
                WS * bg[1536 * g:1536 * (g + 1)]).astype(FP8NP)

    cst8 = np.zeros((P, 2 + 2 * BC), dtype=FP8NP)
    cst8[:, 0:2] = 1.0
    cst8[0, 2:2 + BC] = 1.0  # bias-row rhs: partition 0, j=0 ones
    shared = {
        "W0p": _drpack(W0, FP8NP),
        "W1p": np.ascontiguousarray(
            _drpack(W1, FP8NP).transpose(1, 0, 2, 3)),  # [P, 4, 2, M]
        "W2": (WS * f(inputs["W2"]) * g2).astype(np.float32),
        "W3": (WS * f(inputs["W3"]) * g3).astype(np.float32),
        "Wh0x": np.stack([_drpack(Wh0[g, OUT_B:], FP8NP)
                          for g in range(BLOCKS)]),
        "Wh0d": wh0d,
        "Wh1": wh1,
        "Wgp": np.ascontiguousarray(wgp.transpose(0, 2, 1, 3, 4)),
        "cst": _make_const_block(inputs),
        "cst8": cst8,
    }
    # Wh0x packed as [B, pairs, P, 2, M] -> want [B, P, pairs, 2, M]
    shared["Wh0x"] = np.ascontiguousarray(
        shared["Wh0x"].transpose(0, 2, 1, 3, 4))
    # W0p stays [16, P, 2, M] (indexed by pair in the DMA loop)

    in_maps = []
    for c in range(NCORES):
        sl = slice(c * BC, (c + 1) * BC)
        m = dict(shared)
        dT = np.ascontiguousarray(deter[sl].T)
        m["dT8"] = dT.astype(FP8NP)
        m["dTb"] = dT.astype(_ml.bfloat16)
        m["sT8"] = np.ascontiguousarray(stoch[sl].T).astype(FP8NP)
        m["aT"] = np.ascontiguousarray(action[sl].T)
        m["eT"] = np.ascontiguousarray(d_emb[sl].T)
        in_maps.append(m)
    return in_maps


def _gate_bias_zero(inputs):
    return not np.any(np.asarray(inputs["bg"]))


def _run(inputs, trace=False):
    from concourse import bass_utils
    zb = _gate_bias_zero(inputs)
    nc = _get_program(zb)
    in_maps = _prep_inputs(inputs, zb)
    res = bass_utils.run_bass_kernel_spmd(
        nc, in_maps, core_ids=list(range(NCORES)), trace=trace)
    out = np.empty((B, DETER), dtype=np.float32)
    for c in range(NCORES):
        out[c * BC:(c + 1) * BC, :] = \
            np.asarray(res.results[c]["outT"]).astype(np.float32).T
    return out, res.exec_time_ns


def kernel(**inputs):
    out, _ = _run(inputs, trace=False)
    return out


# ---------------------------------------------------------------------------
# benchmarking helper (test-only; the grading path is kernel() above)
# ---------------------------------------------------------------------------

def _bench_generic(nc, in_maps, iters, n_cores=None):
    """Time repeated device executions with device-resident inputs."""
    import time
    import jax
    import concourse.mybir as mybir
    from jax.sharding import Mesh, NamedSharding, PartitionSpec
    from jax.experimental.shard_map import shard_map
    from concourse import bass2jax

    bass2jax.install_neuronx_cc_hook()
    if n_cores is None:
        n_cores = len(in_maps)

    in_names, out_names, out_avals = [], [], []
    for alloc in nc.m.functions[0].allocations:
        if not isinstance(alloc, mybir.MemoryLocationSet):
            continue
        name = alloc.memorylocations[0].name
        pid_name = (nc.partition_id_tensor.name
                    if nc.partition_id_tensor else None)
        if alloc.kind == "ExternalInput":
            if name != pid_name:
                in_names.append(name)
        elif alloc.kind == "ExternalOutput":
            out_names.append(name)
            out_avals.append(jax.core.ShapedArray(
                tuple(alloc.tensor_shape), mybir.dt.np(alloc.dtype)))
    n_params = len(in_names)

    pid_name = nc.partition_id_tensor.name if nc.partition_id_tensor else None
    bind_names = in_names + out_names + ([pid_name] if pid_name else [])

    def _body(*args):
        operands = list(args)
        if pid_name:
            operands.append(bass2jax.partition_id_tensor())
        outs = bass2jax._bass_exec_p.bind(
            *operands,
            out_avals=tuple(out_avals),
            in_names=tuple(bind_names),
            out_names=tuple(out_names),
            lowering_input_output_aliases=(),
            sim_require_finite=True,
            sim_require_nnan=True,
            nc=nc,
        )
        return tuple(outs)

    devices = jax.devices()[:n_cores]
    mesh = Mesh(np.asarray(devices), ("core",))
    nshard = NamedSharding(mesh, PartitionSpec("core"))
    sharded = jax.jit(
        shard_map(_body, mesh=mesh,
                  in_specs=(PartitionSpec("core"),) * (n_params + len(out_names)),
                  out_specs=(PartitionSpec("core"),) * len(out_names),
                  check_rep=False),
        keep_unused=True)

    concat_in = [
        jax.device_put(
            np.concatenate([np.asarray(in_maps[c][nm]) for c in range(n_cores)],
                           axis=0), nshard)
        for nm in in_names]
    concat_zeros = [
        jax.device_put(
            np.zeros((n_cores * a.shape[0], *a.shape[1:]), a.dtype), nshard)
        for a in out_avals]

    outs = sharded(*concat_in, *concat_zeros)
    jax.block_until_ready(outs)

    BATCH = 6
    diffs = []
    for _ in range(iters):
        t0 = time.perf_counter()
        outs = sharded(*concat_in, *concat_zeros)
        jax.block_until_ready(outs)
        t1 = time.perf_counter()
        for _ in range(BATCH):
            outs = sharded(*concat_in, *concat_zeros)
        jax.block_until_ready(outs)
        t2 = time.perf_counter()
        diffs.append((t2 - t1) - (t1 - t0))
    diffs.sort()
    per_iter_ns = diffs[len(diffs) // 2] / (BATCH - 1) * 1e9
    return outs, per_iter_ns


_TINY = None


def _tiny_program():
    """Near-noop program with the SAME input/output signature, to measure
    axon dispatch overhead differentially."""
    global _TINY
    if _TINY is None:
        nc = bacc.Bacc(trn_type="TRN2", target_bir_lowering=False, debug=False)
        d = {"dT8": ((DETER, BC), fp8), "dTb": ((DETER, BC), bf16),
             "sT8": ((STOCH, BC), fp8), "aT": ((ACT_DIM, BC), f32),
             "eT": ((DEMB, BC), f32), "W0p": ((16, P, 2, HIDDEN), fp8),
             "W1p": ((P, 4, 2, HIDDEN), fp8), "W2": ((ACT_DIM, HIDDEN), f32),
             "W3": ((DEMB, HIDDEN), f32),
             "Wh0x": ((BLOCKS, P, 8, 2, OUT_B), fp8),
             "Wh0d": ((BLOCKS, P, 2, 2, OUT_B), fp8),
             "Wh1": ((BLOCKS, P, 4, OUT_B), bf16),
             "Wgp": ((BLOCKS, P, 2, 2, 3 * OUT_B), fp8),
             "cst": ((P, C_NCOL), f32), "cst8": ((P, 2 + 2 * BC), fp8)}
        aps = {k: nc.dram_tensor(k, list(s), dt, kind="ExternalInput").ap()
               for k, (s, dt) in d.items()}
        outT = nc.dram_tensor("outT", [DETER, BC], bf16,
                              kind="ExternalOutput").ap()
        with tile.TileContext(nc) as tc:
            with tc.tile_pool(name="t", bufs=2) as pool:
                t = pool.tile([P, 4, BC], bf16)
                nc.sync.dma_start(
                    out=t, in_=aps["dTb"][:512, :].rearrange(
                        "(s p) b -> p s b", p=P))
                for g in range(BLOCKS):
                    nc.sync.dma_start(
                        out=outT[512 * g:512 * (g + 1), :].rearrange(
                            "(s p) b -> p s b", p=P),
                        in_=t)
        nc.compile()
        _TINY = nc
    return _TINY


def _bench_overhead(inputs, iters=20):
    nc = _tiny_program()
    in_maps = _prep_inputs(inputs, True)
    _, t = _bench_generic(nc, in_maps, iters)
    return t


def _bench(inputs, iters=20):
    zb = _gate_bias_zero(inputs)
    nc = _get_program(zb)
    in_maps = _prep_inputs(inputs, zb)
    outs, per_iter_ns = _bench_generic(nc, in_maps, iters)
    res = np.asarray(outs[0]).reshape(NCORES, DETER, BC)
    out = np.empty((B, DETER), dtype=np.float32)
    for c in range(NCORES):
        out[c * BC:(c + 1) * BC, :] = res[c].astype(np.float32).T
    return out, per_iter_ns


# revision 42
# speedup vs baseline: 1.0213x; 1.0211x over previous
"""Trainium2 Bass kernel for the Deter GRU-MLP block (RSSM deter update).

Sharding: data-parallel over batch B=4096 across 8 NeuronCores (512 rows
each), all parameters replicated; no collectives.

Design (mixed fp8/bf16, measured 215us TimelineSim vs 407us baseline,
hardware-validated rel-max err 1.42e-2):
- Activations live transposed in SBUF (features on partitions, batch on
  the 512-wide free axis); weights consumed in natural [K, M] layout.
- Big GEMMs run as fp8(e4m3) DoubleRow matmuls (0.5 cyc/row, two k-tiles
  per instruction, host-packed [pairs, 128, 2, M] weights): branch W0/W1,
  both slices of the block-diagonal L0, and the GRU gate GEMM.  L1 stays
  bf16 for accuracy (numpy quantization model: fp8_sim3.py).
- All weights are scaled by 64 on the host (fp8 normal range) with norm
  gains folded in, so pre-norm PSUM values are 64*y.  The rmsnorm absorbs
  the scale: squares are (64y)^2 in bf16, the sqrt fold yields
  1/(64*rms) directly, and the gate nonlinearities use the ACT scale
  operand (sigmoid(acc/64 + b)).
- Per 4-tile norm unit: PSUM->SBUF copies ride the otherwise-idle ACT
  engine (Identity+bias) in L0/L1, squares are a 2x-rate bf16 DVE
  multiply, the sum-of-squares reduction is a ones-vector matmul, and
  silu is a batched ACT sigmoid + DVE multiply casting to fp8/bf16 on
  write.  The four branch norms share two fused sqrt/recip/broadcasts.
- Engines are in-order, so loops are software-pipelined: block g+1's
  normalize+silu is emitted during block g's GEMMs, each block's ss
  matmuls are deferred past the next block's GEMMs, and the first/last
  blocks around each norm barrier run per-tile (or per-half for the gate
  sigmoid/stt/tanh chain) to shorten restart and drain latency.
  Cross-phase weight prefetch tiles live in enclosing pool scopes so
  their DMAs do not inherit false WARs from reused SBUF regions.
- deter is read once as fp8 (branch-0 GEMM + L0 dg slice, resident) and
  once as bf16 (GRU mix operand, streamed during the gates phase);
  output is written bf16 and upcast on the host.
- Hardware-legality notes baked in: gpsimd(Pool) cannot touch PSUM and
  cannot run TensorScalarPtr; DoubleRow Ldweights rejects degenerate
  single-column stationary tiles; Sqrt lives in a different ACT table
  set than Sigmoid/Tanh, so table switches are pre-triggered by tiny
  warm-up ops off the critical path.
"""

import os
import sys
from contextlib import ExitStack

import numpy as np
import ml_dtypes as _ml

for _p in ("/opt/trn_rl_repo", "/opt/pypackages"):
    if os.path.isdir(_p) and _p not in sys.path:
        sys.path.insert(0, _p)

os.environ.setdefault("MYCRO_LOCAL_CACHE", "1")

import concourse.bass as bass  # noqa: E402
import concourse.bacc as bacc  # noqa: E402
import concourse.mybir as mybir  # noqa: E402
import concourse.tile as tile  # noqa: E402

# ---- problem constants (hardcoded; kernel.py must be self-contained) ----
P = 128
B = 4096
NCORES = 8
BC = B // NCORES  # 512 batch columns per core
DETER = 4096
STOCH = 1024
ACT_DIM = 32
DEMB = 16
HIDDEN = 512
BLOCKS = 8
OUT_B = DETER // BLOCKS  # 512
IN_B0 = 4 * HIDDEN + OUT_B  # 2560
EPS = 1e-4

ND = DETER // P       # 32 deter k/n tiles
NX = 4 * HIDDEN // P  # 16 x k tiles
WS = 64.0             # weight scale folded into rmsnorm / gate scales

# const-block column layout (single [P, C_NCOL] f32 DRAM input)
C_BX64 = 0            # 16: 64*(branch bias * gain)
C_BH064 = 16          # 32: 64*(bh0 * gh0)
C_BH164 = 48          # 32: 64*(bh1 * gh1)
C_BG = 80             # 96: bg (unscaled, sigmoid bias)
C_BG64 = 176          # 96: 64*bg (cand stt bias)
C_BGM1 = 272          # 96: bg - 1 (update sigmoid bias)
C_EPSH = 368          # 1: 4096*EPS
C_NEG1 = 369          # 1: -1.0 (update-gate bias)
C_NCOL = 370

f32 = mybir.dt.float32
f32r = mybir.dt.float32r
bf16 = mybir.dt.bfloat16
fp8 = mybir.dt.float8e4

# (L1 is the only bf16 GEMM; see fp8_sim3.py for the quantization study)

_PROG = {}


def _r(ap):
    return ap.bitcast(f32r)


def _build_program(zb):
    """Build the single-core SPMD Bass program (same on all 8 cores).

    zb: gate biases are all zero -> skip the bias K-row in the gate GEMM
    (saves a weight pair per m-tile and 3MB of DMA per core)."""
    AF = mybir.ActivationFunctionType
    Alu = mybir.AluOpType
    DR = mybir.MatmulPerfMode.DoubleRow
    nc = bacc.Bacc(trn_type="TRN2", target_bir_lowering=False, debug=False)

    def din(name, shape, dt=f32):
        return nc.dram_tensor(name, list(shape), dt, kind="ExternalInput").ap()

    dT8 = din("dT8", (DETER, BC), fp8)
    dTb = din("dTb", (DETER, BC), bf16)
    sT8 = din("sT8", (STOCH, BC), fp8)
    aT = din("aT", (ACT_DIM, BC))
    eT = din("eT", (DEMB, BC))
    W0p = din("W0p", (16, P, 2, HIDDEN), fp8)
    W1p = din("W1p", (P, 4, 2, HIDDEN), fp8)
    W2 = din("W2", (ACT_DIM, HIDDEN))
    W3 = din("W3", (DEMB, HIDDEN))
    Wh0x = din("Wh0x", (BLOCKS, P, 8, 2, OUT_B), fp8)
    Wh0d = din("Wh0d", (BLOCKS, P, 2, 2, OUT_B), fp8)
    Wh1 = din("Wh1", (BLOCKS, P, 4, OUT_B), bf16)
    NKP = 2 if zb else 3
    Wgp = din("Wgp", (BLOCKS, P, NKP, 2, 3 * OUT_B), fp8)
    cst = din("cst", (P, C_NCOL))
    cst8 = din("cst8", (P, 2 + 2 * BC), fp8)
    outT = nc.dram_tensor("outT", [DETER, BC], bf16,
                          kind="ExternalOutput").ap()

    with tile.TileContext(nc) as tc, ExitStack() as top:
        consts = top.enter_context(tc.tile_pool(name="consts", bufs=1))
        cst_sb = consts.tile([P, C_NCOL], f32)
        nc.sync.dma_start(out=_r(cst_sb), in_=_r(cst))
        cst8_sb = consts.tile([P, 2 + 2 * BC], fp8)
        nc.sync.dma_start(out=cst8_sb, in_=cst8)
        ones8 = cst8_sb[:, 0:2]
        ones_b16 = consts.tile([P, 1], bf16)
        nc.vector.memset(ones_b16, 1.0)
        onesp = cst8_sb[:, 2:2 + 2 * BC].rearrange("p (j b) -> p j b", j=2)
        bgm1 = cst_sb[:, C_BGM1:C_BGM1 + 96]
        epsh = cst_sb[:1, C_EPSH:C_EPSH + 1]
        neg1 = cst_sb[:, C_NEG1:C_NEG1 + 1]

        # resident regions
        mainp = top.enter_context(tc.tile_pool(name="mainp", bufs=1))
        main_sb = mainp.tile([P, ND, BC], bf16)   # pre-norm y64 / h0n
        dtbp = top.enter_context(tc.tile_pool(name="dtbp", bufs=1))
        dtb_sb = dtbp.tile([P, ND, BC], bf16)     # deter bf16 (L0 dg + mix)
        x8p = top.enter_context(tc.tile_pool(name="x8p", bufs=1))
        x8_sb = x8p.tile([P, NX, BC], fp8)        # branch outputs (L0 rhs)
        h1p = top.enter_context(tc.tile_pool(name="h1p", bufs=1))
        h1n8 = h1p.tile([P, ND, BC], fp8)         # L1 normalized (gates rhs)

        ysqp = top.enter_context(tc.tile_pool(name="ysqp", bufs=3))
        invp = top.enter_context(tc.tile_pool(name="invp", bufs=2))
        invbp = top.enter_context(tc.tile_pool(name="invbp", bufs=2))
        sigp = top.enter_context(tc.tile_pool(name="sigp", bufs=3))

        def act_warm(func, name):
            """Trigger an ACT table switch off the critical path."""
            t = invp.tile([1, 1], f32, tag="warm", name=f"warm_{name}")
            nc.scalar.activation(out=t, in_=epsh, func=func)

        def finish_norm(ss_flat, D, width, name):
            """invb = 1/(64*sqrt(ss/D + eps)), bf16, broadcast to all
            partitions.  ss_flat: [1, width*BC] (PSUM)."""
            sq = invp.tile([1, width * BC], f32, tag="sq", name=f"sq_{name}")
            nc.scalar.activation(out=sq, in_=ss_flat, func=AF.Sqrt,
                                 bias=epsh, scale=1.0 / D)
            act_warm(AF.Sigmoid, f"sg_{name}")  # reload hidden under recip
            inv1 = invp.tile([1, width * BC], bf16, tag="inv1",
                             name=f"inv1_{name}")
            with nc.allow_low_precision(reason="bf16 rstd broadcast"):
                nc.vector.reciprocal(inv1, sq)
            invb = invbp.tile([P, width * BC], bf16, tag="invb",
                              name=f"invb_{name}")
            nc.gpsimd.partition_broadcast(invb, inv1)
            return invb

        def norm_silu_unit(unit_y, invb, out_unit, name, per_tile=False,
                           mul_pool=False):
            """out_unit <- silu(unit_y * invb) = t*sigmoid(t), t=y*inv.
            per_tile=True pipelines at tile granularity (lower latency
            right after a norm barrier)."""
            mul_eng = nc.gpsimd if mul_pool else nc.vector
            for m in range(4):
                nc.vector.tensor_mul(unit_y[:, m, :], unit_y[:, m, :],
                                     invb[:, m * BC:(m + 1) * BC]
                                     if invb.shape[-1] == 4 * BC else invb)
            s = sigp.tile([P, 4, BC], bf16, tag="sig", name=f"sig_{name}")
            if per_tile:
                for m in range(4):
                    nc.scalar.activation(out=s[:, m, :], in_=unit_y[:, m, :],
                                         func=AF.Sigmoid)
                    mul_eng.tensor_mul(out_unit[:, m, :], unit_y[:, m, :],
                                       s[:, m, :])
            else:
                nc.scalar.activation(
                    out=s.rearrange("p a b -> p (a b)"),
                    in_=unit_y.rearrange("p a b -> p (a b)"), func=AF.Sigmoid)
                mul_eng.tensor_mul(
                    out_unit.rearrange("p a b -> p (a b)"),
                    unit_y.rearrange("p a b -> p (a b)"),
                    s.rearrange("p a b -> p (a b)"))

        # ============ phases A, L0, L1 (shared PSUM layout) ============
        with ExitStack() as ph_al:
            psum_acc = ph_al.enter_context(
                tc.tile_pool(name="pacc", bufs=6, space="PSUM"))
            psum_ss = ph_al.enter_context(
                tc.tile_pool(name="pss", bufs=1, space="PSUM"))
            d8pool = ph_al.enter_context(tc.tile_pool(name="d8r", bufs=1))
            d8sb = d8pool.tile([P, ND, BC], fp8)  # fp8 deter (br0 + L0 dg)

            def unit_post(unit_y, accs, b64col0, ss, ss_first, ss_last,
                          name, act_copy, sq_pool=False, presum=False):
                """copy accs (+64*bias) into unit_y (bf16) -- on ACT
                (Identity) when act_copy else DVE; squares (64y)^2 bf16.
                Returns a closure emitting the 4 ss ones-matmuls, so the
                caller can defer them past the next block's GEMMs (the PE
                is in-order; immediate ss would stall it on this block's
                elementwise post)."""
                for m in range(4):
                    bcol = cst_sb[:, b64col0 + m:b64col0 + m + 1]
                    if act_copy and (act_copy == 2 or m % 2 == 0):
                        nc.scalar.activation(out=unit_y[:, m, :],
                                             in_=accs[m], func=AF.Identity,
                                             bias=bcol)
                    else:
                        nc.vector.tensor_scalar_add(unit_y[:, m, :],
                                                    accs[m], bcol)
                ysq = ysqp.tile([P, 4, BC], bf16, tag="ysq",
                                name=f"ysq_{name}")
                if sq_pool:
                    for half in range(2):
                        seg = unit_y[:, 2 * half:2 * half + 2, :].rearrange(
                            "p a b -> p (a b)")
                        nc.gpsimd.tensor_mul(
                            ysq[:, 2 * half:2 * half + 2, :].rearrange(
                                "p a b -> p (a b)"), seg, seg)
                else:
                    for t in range(4):
                        nc.vector.tensor_mul(ysq[:, t, :], unit_y[:, t, :],
                                             unit_y[:, t, :])

                if presum:
                    # fold 4 tiles into 1 on the (idle) DVE; the PE then
                    # does a single ones-matmul instead of four
                    nc.vector.tensor_add(ysq[:, 0, :], ysq[:, 0, :],
                                         ysq[:, 1, :])
                    nc.vector.tensor_add(ysq[:, 2, :], ysq[:, 2, :],
                                         ysq[:, 3, :])
                    nc.vector.tensor_add(ysq[:, 0, :], ysq[:, 0, :],
                                         ysq[:, 2, :])

                    def emit_ss():
                        nc.tensor.matmul(ss, lhsT=ones_b16, rhs=ysq[:, 0, :],
                                         start=ss_first, stop=ss_last)
                else:
                    def emit_ss():
                        for t in range(4):
                            nc.tensor.matmul(
                                ss, lhsT=ones_b16, rhs=ysq[:, t, :],
                                start=(ss_first and t == 0),
                                stop=(ss_last and t == 3))
                return emit_ss

            # ---------------- phase A: four input branches ----------------
            with ExitStack() as ph_a:
                w0p_pool = ph_a.enter_context(
                    tc.tile_pool(name="w0p", bufs=8))
                sp = ph_a.enter_context(tc.tile_pool(name="sp", bufs=1))

                sT_sb = sp.tile([P, STOCH // P, BC], fp8)
                aT_sb = sp.tile([ACT_DIM, BC], f32)
                eT_sb = sp.tile([DEMB, BC], f32)
                an_sb = sp.tile([ACT_DIM, BC], f32)
                w3t = sp.tile([DEMB, HIDDEN], f32)
                w2t = sp.tile([ACT_DIM, HIDDEN], f32)
                w1t = sp.tile([P, 4, 2, HIDDEN], fp8)

                # the 4MB branch-0 stream is the phase-A long pole: first
                w0_slabs = []
                for c in range(8):
                    nc.sync.dma_start(
                        out=d8sb[:, 4 * c:4 * c + 4, :],
                        in_=dT8[512 * c:512 * (c + 1), :].rearrange(
                            "(s p) b -> p s b", p=P))
                    w0 = w0p_pool.tile([P, 2, 2, HIDDEN], fp8, tag="w0",
                                       name=f"w0_{c}")
                    nc.sync.dma_start(
                        out=w0, in_=W0p[2 * c:2 * c + 2].rearrange(
                            "s p j m -> p s j m"))
                    w0_slabs.append(w0)
                    if c == 0:
                        nc.sync.dma_start(out=_r(eT_sb), in_=_r(eT))
                        nc.sync.dma_start(out=_r(w3t), in_=_r(W3))
                        nc.sync.dma_start(out=aT_sb, in_=aT)
                        nc.sync.dma_start(out=_r(w2t), in_=_r(W2))
                    if c == 1:
                        nc.sync.dma_start(
                            out=sT_sb,
                            in_=sT8.rearrange("(s p) b -> p s b", p=P))
                        nc.sync.dma_start(out=w1t, in_=W1p)

                # action preprocess: a / max(|a|, 1)
                ab = sp.tile([ACT_DIM, BC], f32)
                nc.scalar.activation(out=ab, in_=aT_sb, func=AF.Abs)
                act_warm(AF.Sqrt, "br")
                nc.vector.tensor_scalar_max(ab, ab, 1.0)
                nc.vector.reciprocal(ab, ab)
                nc.vector.tensor_mul(_r(an_sb), aT_sb, ab)

                def branch_small(br, wt, rhs):
                    accs = []
                    for m in range(4):
                        acc = psum_acc.tile([P, BC], f32, tag="acc",
                                            name=f"acc_br{br}_{m}")
                        nc.tensor.matmul(acc,
                                         lhsT=_r(wt[:, m * P:(m + 1) * P]),
                                         rhs=_r(rhs), start=True, stop=True)
                        accs.append(acc)
                    return accs

                def branch_dr(br, npairs, wslab, rhs_pair):
                    accs = [psum_acc.tile([P, BC], f32, tag="acc",
                                          name=f"acc_br{br}_{m}")
                            for m in range(4)]
                    for kp in range(npairs):
                        w = wslab(kp)
                        rhs = rhs_pair(kp)
                        for m in range(4):
                            nc.tensor.matmul(
                                accs[m], lhsT=w[:, :, m * P:(m + 1) * P],
                                rhs=rhs, start=(kp == 0),
                                stop=(kp == npairs - 1), perf_mode=DR)
                    return accs

                # branches 1-3 share one fused norm (single sqrt/recip/
                # broadcast over 3 stacked ss slots); branch 0 -- whose GEMM
                # is gated on the big dT8/W0p DMA stream -- normalizes on
                # its own so the other three silus overlap that stream.
                ss32 = [psum_ss.tile([1, BC], f32, tag="ss", name=f"ss_b{b}")
                        for b in (3, 2)]

                def branch_post(br, accs, ss):
                    unit = main_sb[:, 4 * br:4 * br + 4, :]
                    unit_post(unit, accs, C_BX64 + 4 * br, ss,
                              True, True, f"br{br}", act_copy=2,
                              sq_pool=(br == 3))()

                branch_post(3, branch_small(3, w3t, eT_sb), ss32[0])
                branch_post(2, branch_small(2, w2t, an_sb), ss32[1])
                ssb1 = psum_ss.tile([1, BC], f32, tag="ss", name="ssb1")
                branch_post(1, branch_dr(
                    1, 4, lambda kp: w1t[:, kp, :, :],
                    lambda kp: sT_sb[:, 2 * kp:2 * kp + 2, :]), ssb1)
                for i, br in enumerate((3, 2)):
                    invb = finish_norm(ss32[i], HIDDEN, 1, f"br{br}")
                    norm_silu_unit(
                        main_sb[:, 4 * br:4 * br + 4, :], invb,
                        x8_sb[:, 4 * br:4 * br + 4, :], f"br{br}",
                        mul_pool=(br == 3))
                invb1b = finish_norm(ssb1, HIDDEN, 1, "br1")
                norm_silu_unit(main_sb[:, 4:8, :], invb1b,
                               x8_sb[:, 4:8, :], "br1")

                ssb0 = psum_ss.tile([1, BC], f32, tag="ss", name="ssb0")
                branch_post(0, branch_dr(
                    0, 16, lambda kp: w0_slabs[kp // 2][:, kp % 2, :, :],
                    lambda kp: d8sb[:, 2 * kp:2 * kp + 2, :]), ssb0)
                invb0b = finish_norm(ssb0, HIDDEN, 1, "br0")
                norm_silu_unit(main_sb[:, 0:4, :], invb0b,
                               x8_sb[:, 0:4, :], "br0", per_tile=True)

            # ---------- L0: BlockLinear(2560 -> 512/block) ----------
            with ExitStack() as ph_l:
                wh0xp = ph_l.enter_context(
                    tc.tile_pool(name="wh0xp", bufs=2))
                wh0dp = ph_l.enter_context(
                    tc.tile_pool(name="wh0dp", bufs=2))
                wh1p = ph_l.enter_context(tc.tile_pool(name="wh1p", bufs=3))

                ss0 = psum_ss.tile([1, BC], f32, tag="ss", name="ss_l0")
                pend0 = []
                for g in range(BLOCKS):
                    wd = wh0dp.tile([P, 2, 2, OUT_B], fp8, tag="wh0d",
                                    name=f"wh0d_{g}")
                    nc.sync.dma_start(out=wd, in_=Wh0d[g])
                    wx = wh0xp.tile([P, 8, 2, OUT_B], fp8, tag="wh0x",
                                    name=f"wh0x_{g}")
                    nc.sync.dma_start(out=wx, in_=Wh0x[g])
                    accs = [psum_acc.tile([P, BC], f32, tag="acc",
                                          name=f"acc_h0_{g}_{m}")
                            for m in range(4)]
                    # dg (fp8 DR) matmuls for all m first: they only need
                    # the resident fp8 deter + wd, so the PE can run them
                    # while the branch silu (x8) is still finishing
                    for m in range(4):
                        for kp in range(2):
                            nc.tensor.matmul(
                                accs[m],
                                lhsT=wd[:, kp, :, m * P:(m + 1) * P],
                                rhs=d8sb[:, 4 * g + 2 * kp:4 * g + 2 * kp + 2,
                                         :],
                                start=(kp == 0), stop=False, perf_mode=DR)
                    for m in range(4):
                        for kp in (2, 3, 4, 5, 6, 7, 0, 1):
                            nc.tensor.matmul(
                                accs[m], lhsT=wx[:, kp, :, m * P:(m + 1) * P],
                                rhs=x8_sb[:, 2 * kp:2 * kp + 2, :],
                                start=False, stop=(kp == 1), perf_mode=DR)
                    # deferred ss for the previous block: keeps the PE from
                    # stalling on this block's elementwise post
                    if pend0:
                        pend0.pop()()
                    pend0.append(unit_post(
                        main_sb[:, 4 * g:4 * g + 4, :], accs,
                        C_BH064 + 4 * g, ss0, g == 0, g == BLOCKS - 1,
                        f"l0_{g}", act_copy=True, sq_pool=(g % 2 == 0)))
                    if g == 5:
                        act_warm(AF.Sqrt, "l0")
                pend0.pop()()
                invb0 = finish_norm(ss0, DETER, 1, "l0")

                # --------- L1 (bf16), interleaved with the L0 norm ---------
                ss1 = psum_ss.tile([1, BC], f32, tag="ss", name="ss_l1")
                # h0n = silu(norm(h0)) in place; block 0 primed per-tile,
                # block g+1's silu is emitted during block g (in-order
                # engines would otherwise serialize consecutive blocks)
                norm_silu_unit(main_sb[:, 0:4, :], invb0,
                               main_sb[:, 0:4, :], "l1_0", per_tile=True)
                pend1 = []
                for g in range(BLOCKS):
                    unit = main_sb[:, 4 * g:4 * g + 4, :]
                    # stream the bf16 deter (GRU mix operand) here, where
                    # the DMA engines are otherwise idle
                    nc.sync.dma_start(
                        out=dtb_sb[:, 4 * g:4 * g + 4, :],
                        in_=dTb[512 * g:512 * (g + 1), :].rearrange(
                            "(s p) b -> p s b", p=P))
                    w1h = wh1p.tile([P, 4, OUT_B], bf16, tag="wh1",
                                    name=f"wh1_{g}")
                    nc.sync.dma_start(out=w1h, in_=Wh1[g])
                    accs = [psum_acc.tile([P, BC], f32, tag="acc",
                                          name=f"acc_h1_{g}_{m}")
                            for m in range(4)]
                    for m in range(4):
                        for kk in range(4):
                            nc.tensor.matmul(
                                accs[m], lhsT=w1h[:, kk, m * P:(m + 1) * P],
                                rhs=unit[:, kk, :],
                                start=(kk == 0), stop=(kk == 3))
                    if g + 1 < BLOCKS:
                        nxt = main_sb[:, 4 * (g + 1):4 * (g + 1) + 4, :]
                        norm_silu_unit(nxt, invb0, nxt, f"l1_{g + 1}")
                    if pend1:
                        pend1.pop()()
                    pend1.append(unit_post(
                        unit, accs, C_BH164 + 4 * g, ss1,
                        g == 0, g == BLOCKS - 1, f"l1_{g}",
                        act_copy=True, sq_pool=(g % 2 == 0)))
                    if g == 5:
                        act_warm(AF.Sqrt, "l1")
                pend1.pop()()
                invb1 = finish_norm(ss1, DETER, 1, "l1")

        # ------------- GRU gates + final mix (per block) -------------
        with ExitStack() as ph_g:
            gpsum = ph_g.enter_context(
                tc.tile_pool(name="gpsum", bufs=2, space="PSUM"))
            wgp = ph_g.enter_context(tc.tile_pool(name="wgpool", bufs=3))
            grup = ph_g.enter_context(tc.tile_pool(name="grup", bufs=3))

            norm_silu_unit(main_sb[:, 0:4, :], invb1, h1n8[:, 0:4, :],
                           "g0", per_tile=True)
            for g in range(BLOCKS):
                wg = wgp.tile([P, NKP, 2, 3 * OUT_B], fp8, tag="wg",
                              name=f"wg_{g}")
                nc.sync.dma_start(out=wg, in_=Wgp[g])
                r_sb = grup.tile([P, 4, BC], bf16, tag="r", name=f"r_{g}")
                c_sb = grup.tile([P, 4, BC], bf16, tag="c", name=f"c_{g}")
                u_sb = grup.tile([P, 4, BC], bf16, tag="u", name=f"u_{g}")

                def gate_group(grp):
                    """12 DoubleRow matmuls (2 data pairs + bias pair) into
                    a 4-bank PSUM group for gate third grp."""
                    acc4 = gpsum.tile([P, 4, BC], f32, tag="g4",
                                      name=f"acc_g{g}_{grp}")
                    for m in range(4):
                        mm = 4 * grp + m
                        for kp in range(NKP):
                            rhs = (onesp if kp == 2 else
                                   h1n8[:, 4 * g + 2 * kp:4 * g + 2 * kp + 2, :])
                            nc.tensor.matmul(
                                acc4[:, m, :],
                                lhsT=wg[:, kp, :, mm * P:(mm + 1) * P],
                                rhs=rhs, start=(kp == 0),
                                stop=(kp == NKP - 1), perf_mode=DR)
                    return acc4

                accr = gate_group(0)
                if g == BLOCKS - 1:
                    for m in range(4):
                        nc.scalar.activation(
                            out=r_sb[:, m, :], in_=accr[:, m, :],
                            func=AF.Sigmoid, scale=1.0 / WS)
                else:
                    for h in range(2):
                        nc.scalar.activation(
                            out=r_sb[:, 2 * h:2 * h + 2, :].rearrange(
                                "p a b -> p (a b)"),
                            in_=accr[:, 2 * h:2 * h + 2, :].rearrange(
                                "p a b -> p (a b)"),
                            func=AF.Sigmoid, scale=1.0 / WS)
                if g + 1 < BLOCKS:
                    nxt = main_sb[:, 4 * (g + 1):4 * (g + 1) + 4, :]
                    norm_silu_unit(nxt, invb1,
                                   h1n8[:, 4 * (g + 1):4 * (g + 1) + 4, :],
                                   f"g{g + 1}", mul_pool=False)
                accc = gate_group(1)
                if g == BLOCKS - 1:
                    for m in range(4):
                        nc.vector.scalar_tensor_tensor(
                            out=c_sb[:, m, :], in0=accc[:, m, :],
                            scalar=1.0 / WS, in1=r_sb[:, m, :],
                            op0=Alu.mult, op1=Alu.mult)
                        nc.scalar.activation(out=c_sb[:, m, :],
                                             in_=c_sb[:, m, :], func=AF.Tanh)
                else:
                    for h in range(2):
                        seg = c_sb[:, 2 * h:2 * h + 2, :].rearrange(
                            "p a b -> p (a b)")
                        nc.vector.scalar_tensor_tensor(
                            out=seg,
                            in0=accc[:, 2 * h:2 * h + 2, :].rearrange(
                                "p a b -> p (a b)"),
                            scalar=1.0 / WS,
                            in1=r_sb[:, 2 * h:2 * h + 2, :].rearrange(
                                "p a b -> p (a b)"),
                            op0=Alu.mult, op1=Alu.mult)
                        nc.scalar.activation(out=seg, in_=seg, func=AF.Tanh)
                accu = gate_group(2)
                if g == BLOCKS - 1:
                    for m in range(4):
                        nc.scalar.activation(
                            out=u_sb[:, m, :], in_=accu[:, m, :],
                            func=AF.Sigmoid, scale=1.0 / WS, bias=neg1)
                else:
                    nc.scalar.activation(
                        out=u_sb.rearrange("p a b -> p (a b)"),
                        in_=accu.rearrange("p a b -> p (a b)"),
                        func=AF.Sigmoid, scale=1.0 / WS, bias=neg1)

                if g < BLOCKS - 1:
                    cflat = c_sb.rearrange("p a b -> p (a b)")
                    dunit = dtb_sb[:, 4 * g:4 * g + 4, :].rearrange(
                        "p a b -> p (a b)")
                    uflat = u_sb.rearrange("p a b -> p (a b)")
                    # out = d + u*(c-d), in place in c_sb: sub alternates
                    # Pool/DVE by block parity, mul+add on DVE
                    sub_eng = nc.vector
                    sub_eng.tensor_sub(cflat, cflat, dunit)
                    nc.vector.tensor_mul(cflat, uflat, cflat)
                    nc.vector.tensor_add(cflat, dunit, cflat)
                    nc.sync.dma_start(
                        out=outT[512 * g:512 * (g + 1), :].rearrange(
                            "(s p) b -> p s b", p=P),
                        in_=c_sb)
                else:
                    # last block: per-tile mix so the out DMA overlaps
                    for m in range(4):
                        ct = c_sb[:, m, :]
                        dt_ = dtb_sb[:, 4 * g + m, :]
                        nc.vector.tensor_sub(ct, ct, dt_)
                        nc.vector.tensor_mul(ct, u_sb[:, m, :], ct)
                        nc.vector.tensor_add(ct, dt_, ct)
                        nc.sync.dma_start(
                            out=outT[512 * g + P * m:512 * g + P * (m + 1),
                                     :],
                            in_=ct)

    nc.compile()
    return nc


def _get_program(zb=True):
    if zb not in _PROG:
        _PROG[zb] = _build_program(zb)
    return _PROG[zb]


FP8NP = _ml.float8_e4m3


def _drpack(W, dt):
    """[K, M] -> [K//256, 128, 2, M] DoubleRow-packed, cast to dt."""
    K, M = W.shape
    return np.ascontiguousarray(
        W.reshape(K // 256, 2, P, M).transpose(0, 2, 1, 3)).astype(dt)


def _kpack(W, dt):
    """[K, M] -> [128, K//128, M] (plain k-tiled lhsT), cast to dt."""
    K, M = W.shape
    return np.ascontiguousarray(
        W.reshape(K // P, P, M).transpose(1, 0, 2)).astype(dt)


def _make_const_block(inputs):
    f = lambda a: np.asarray(a, dtype=np.float32)
    cst = np.zeros((P, C_NCOL), dtype=np.float32)
    cst[:, C_BX64:C_BX64 + 16] = WS * np.stack(
        [f(inputs[b]) * f(inputs[g]) for b, g in
         (("b0", "g0"), ("b1", "g1"), ("b2", "g2"), ("b3", "g3"))]
    ).reshape(16, P).T
    cst[:, C_BH064:C_BH064 + 32] = WS * (
        f(inputs["bh0"]) * f(inputs["gh0"])).reshape(32, P).T
    bgt = f(inputs["bg"]).reshape(96, P).T
    cst[:, C_BH164:C_BH164 + 32] = WS * (
        f(inputs["bh1"]) * f(inputs["gh1"])).reshape(32, P).T
    cst[:, C_BG:C_BG + 96] = bgt
    cst[:, C_BG64:C_BG64 + 96] = WS * bgt
    cst[:, C_BGM1:C_BGM1 + 96] = bgt - 1.0
    cst[:, C_EPSH] = WS * WS * EPS
    cst[:, C_NEG1] = -1.0
    return cst


def _prep_inputs(inputs, zb=True):
    """Host-side shard + transpose + quantized weight packing."""
    f = lambda a: np.ascontiguousarray(np.asarray(a), dtype=np.float32)
    stoch = f(inputs["stoch"]).reshape(B, -1)
    deter = f(inputs["deter"])
    action = f(inputs["action"])
    d_emb = f(inputs["d_emb"])

    g0, g1 = f(inputs["g0"]), f(inputs["g1"])
    g2, g3 = f(inputs["g2"]), f(inputs["g3"])
    gh0, gh1 = f(inputs["gh0"]), f(inputs["gh1"])

    W0 = WS * f(inputs["W0"]) * g0
    W1 = WS * f(inputs["W1"]) * g1
    Wh0 = WS * f(inputs["Wh0"]) * gh0.reshape(BLOCKS, 1, OUT_B)
    Wh1 = WS * f(inputs["Wh1"]) * gh1.reshape(BLOCKS, 1, OUT_B)
    Wg = WS * f(inputs["Wg"])

    wh0d = np.ascontiguousarray(np.stack(
        [_drpack(Wh0[g, :OUT_B], FP8NP) for g in range(BLOCKS)]
    ).transpose(0, 2, 1, 3, 4))  # [B, P, 2, 2, M]
    wh1 = np.stack([_kpack(Wh1[g], _ml.bfloat16) for g in range(BLOCKS)])

    bg = f(inputs["bg"])  # [3*DETER], block g segment [1536g:1536(g+1)]
    nkp = 2 if zb else 3
    wgp = np.zeros((BLOCKS, nkp, P, 2, 3 * OUT_B), dtype=FP8NP)
    for g in range(BLOCKS):
        wgp[g, :2] = _drpack(Wg[g], FP8NP)
        if not zb:
            wgp[g, 2, 0, 0, :] = (
                WS * bg[1536 * g:1536 * (g + 1)]).astype(FP8NP)

    cst8 = np.zeros((P, 2 + 2 * BC), dtype=FP8NP)
    cst8[:, 0:2] = 1.0
    cst8[0, 2:2 + BC] = 1.0  # bias-row rhs: partition 0, j=0 ones
    shared = {
        "W0p": _drpack(W0, FP8NP),
        "W1p": np.ascontiguousarray(
            _drpack(W1, FP8NP).transpose(1, 0, 2, 3)),  # [P, 4, 2, M]
        "W2": (WS * f(inputs["W2"]) * g2).astype(np.float32),
        "W3": (WS * f(inputs["W3"]) * g3).astype(np.float32),
        "Wh0x": np.stack([_drpack(Wh0[g, OUT_B:], FP8NP)
                          for g in range(BLOCKS)]),
        "Wh0d": wh0d,
        "Wh1": wh1,
        "Wgp": np.ascontiguousarray(wgp.transpose(0, 2, 1, 3, 4)),
        "cst": _make_const_block(inputs),
        "cst8": cst8,
    }
    # Wh0x packed as [B, pairs, P, 2, M] -> want [B, P, pairs, 2, M]
    shared["Wh0x"] = np.ascontiguousarray(
        shared["Wh0x"].transpose(0, 2, 1, 3, 4))
    # W0p stays [16, P, 2, M] (indexed by pair in the DMA loop)

    in_maps = []
    for c in range(NCORES):
        sl = slice(c * BC, (c + 1) * BC)
        m = dict(shared)
        dT = np.ascontiguousarray(deter[sl].T)
        m["dT8"] = dT.astype(FP8NP)
        m["dTb"] = dT.astype(_ml.bfloat16)
        m["sT8"] = np.ascontiguousarray(stoch[sl].T).astype(FP8NP)
        m["aT"] = np.ascontiguousarray(action[sl].T)
        m["eT"] = np.ascontiguousarray(d_emb[sl].T)
        in_maps.append(m)
    return in_maps


def _gate_bias_zero(inputs):
    return not np.any(np.asarray(inputs["bg"]))


def _run(inputs, trace=False):
    from concourse import bass_utils
    zb = _gate_bias_zero(inputs)
    nc = _get_program(zb)
    in_maps = _prep_inputs(inputs, zb)
    res = bass_utils.run_bass_kernel_spmd(
        nc, in_maps, core_ids=list(range(NCORES)), trace=trace)
    out = np.empty((B, DETER), dtype=np.float32)
    for c in range(NCORES):
        out[c * BC:(c + 1) * BC, :] = \
            np.asarray(res.results[c]["outT"]).astype(np.float32).T
    return out, res.exec_time_ns


def kernel(**inputs):
    out, _ = _run(inputs, trace=False)
    return out


# ---------------------------------------------------------------------------
# benchmarking helper (test-only; the grading path is kernel() above)
# ---------------------------------------------------------------------------

def _bench_generic(nc, in_maps, iters, n_cores=None):
    """Time repeated device executions with device-resident inputs."""
    import time
    import jax
    import concourse.mybir as mybir
    from jax.sharding import Mesh, NamedSharding, PartitionSpec
    from jax.experimental.shard_map import shard_map
    from concourse import bass2jax

    bass2jax.install_neuronx_cc_hook()
    if n_cores is None:
        n_cores = len(in_maps)

    in_names, out_names, out_avals = [], [], []
    for alloc in nc.m.functions[0].allocations:
        if not isinstance(alloc, mybir.MemoryLocationSet):
            continue
        name = alloc.memorylocations[0].name
        pid_name = (nc.partition_id_tensor.name
                    if nc.partition_id_tensor else None)
        if alloc.kind == "ExternalInput":
            if name != pid_name:
                in_names.append(name)
        elif alloc.kind == "ExternalOutput":
            out_names.append(name)
            out_avals.append(jax.core.ShapedArray(
                tuple(alloc.tensor_shape), mybir.dt.np(alloc.dtype)))
    n_params = len(in_names)

    pid_name = nc.partition_id_tensor.name if nc.partition_id_tensor else None
    bind_names = in_names + out_names + ([pid_name] if pid_name else [])

    def _body(*args):
        operands = list(args)
        if pid_name:
            operands.append(bass2jax.partition_id_tensor())
        outs = bass2jax._bass_exec_p.bind(
            *operands,
            out_avals=tuple(out_avals),
            in_names=tuple(bind_names),
            out_names=tuple(out_names),
            lowering_input_output_aliases=(),
            sim_require_finite=True,
            sim_require_nnan=True,
            nc=nc,
        )
        return tuple(outs)

    devices = jax.devices()[:n_cores]
    mesh = Mesh(np.asarray(devices), ("core",))
    nshard = NamedSharding(mesh, PartitionSpec("core"))
    sharded = jax.jit(
        shard_map(_body, mesh=mesh,
                  in_specs=(PartitionSpec("core"),) * (n_params + len(out_names)),
                  out_specs=(PartitionSpec("core"),) * len(out_names),
                  check_rep=False),
        keep_unused=True)

    concat_in = [
        jax.device_put(
            np.concatenate([np.asarray(in_maps[c][nm]) for c in range(n_cores)],
                           axis=0), nshard)
        for nm in in_names]
    concat_zeros = [
        jax.device_put(
            np.zeros((n_cores * a.shape[0], *a.shape[1:]), a.dtype), nshard)
        for a in out_avals]

    outs = sharded(*concat_in, *concat_zeros)
    jax.block_until_ready(outs)

    BATCH = 6
    diffs = []
    for _ in range(iters):
        t0 = time.perf_counter()
        outs = sharded(*concat_in, *concat_zeros)
        jax.block_until_ready(outs)
        t1 = time.perf_counter()
        for _ in range(BATCH):
            outs = sharded(*concat_in, *concat_zeros)
        jax.block_until_ready(outs)
        t2 = time.perf_counter()
        diffs.append((t2 - t1) - (t1 - t0))
    diffs.sort()
    per_iter_ns = diffs[len(diffs) // 2] / (BATCH - 1) * 1e9
    return outs, per_iter_ns


_TINY = None


def _tiny_program():
    """Near-noop program with the SAME input/output signature, to measure
    axon dispatch overhead differentially."""
    global _TINY
    if _TINY is None:
        nc = bacc.Bacc(trn_type="TRN2", target_bir_lowering=False, debug=False)
        d = {"dT8": ((DETER, BC), fp8), "dTb": ((DETER, BC), bf16),
             "sT8": ((STOCH, BC), fp8), "aT": ((ACT_DIM, BC), f32),
             "eT": ((DEMB, BC), f32), "W0p": ((16, P, 2, HIDDEN), fp8),
             "W1p": ((P, 4, 2, HIDDEN), fp8), "W2": ((ACT_DIM, HIDDEN), f32),
             "W3": ((DEMB, HIDDEN), f32),
             "Wh0x": ((BLOCKS, P, 8, 2, OUT_B), fp8),
             "Wh0d": ((BLOCKS, P, 2, 2, OUT_B), fp8),
             "Wh1": ((BLOCKS, P, 4, OUT_B), bf16),
             "Wgp": ((BLOCKS, P, 2, 2, 3 * OUT_B), fp8),
             "cst": ((P, C_NCOL), f32), "cst8": ((P, 2 + 2 * BC), fp8)}
        aps = {k: nc.dram_tensor(k, list(s), dt, kind="ExternalInput").ap()
               for k, (s, dt) in d.items()}
        outT = nc.dram_tensor("outT", [DETER, BC], bf16,
                              kind="ExternalOutput").ap()
        with tile.TileContext(nc) as tc:
            with tc.tile_pool(name="t", bufs=2) as pool:
                t = pool.tile([P, 4, BC], bf16)
                nc.sync.dma_start(
                    out=t, in_=aps["dTb"][:512, :].rearrange(
                        "(s p) b -> p s b", p=P))
                for g in range(BLOCKS):
                    nc.sync.dma_start(
                        out=outT[512 * g:512 * (g + 1), :].rearrange(
                            "(s p) b -> p s b", p=P),
                        in_=t)
        nc.compile()
        _TINY = nc
    return _TINY


def _bench_overhead(inputs, iters=20):
    nc = _tiny_program()
    in_maps = _prep_inputs(inputs, True)
    _, t = _bench_generic(nc, in_maps, iters)
    return t


def _bench(inputs, iters=20):
    zb = _gate_bias_zero(inputs)
    nc = _get_program(zb)
    in_maps = _prep_inputs(inputs, zb)
    outs, per_iter_ns = _bench_generic(nc, in_maps, iters)
    res = np.asarray(outs[0]).reshape(NCORES, DETER, BC)
    out = np.empty((B, DETER), dtype=np.float32)
    for c in range(NCORES):
        out[c * BC:(c + 1) * BC, :] = res[c].astype(np.float32).T
    return out, per_iter_ns


# revision 43
# speedup vs baseline: 1.0217x; 1.0003x over previous
"""Trainium2 Bass kernel for the Deter GRU-MLP block (RSSM deter update).

Sharding: data-parallel over batch B=4096 across 8 NeuronCores (512 rows
each), all parameters replicated; no collectives.

Design (mixed fp8/bf16, measured 215us TimelineSim vs 407us baseline,
hardware-validated rel-max err 1.42e-2):
- Activations live transposed in SBUF (features on partitions, batch on
  the 512-wide free axis); weights consumed in natural [K, M] layout.
- Big GEMMs run as fp8(e4m3) DoubleRow matmuls (0.5 cyc/row, two k-tiles
  per instruction, host-packed [pairs, 128, 2, M] weights): branch W0/W1,
  both slices of the block-diagonal L0, and the GRU gate GEMM.  L1 stays
  bf16 for accuracy (numpy quantization model: fp8_sim3.py).
- All weights are scaled by 64 on the host (fp8 normal range) with norm
  gains folded in, so pre-norm PSUM values are 64*y.  The rmsnorm absorbs
  the scale: squares are (64y)^2 in bf16, the sqrt fold yields
  1/(64*rms) directly, and the gate nonlinearities use the ACT scale
  operand (sigmoid(acc/64 + b)).
- Per 4-tile norm unit: PSUM->SBUF copies ride the otherwise-idle ACT
  engine (Identity+bias) in L0/L1, squares are a 2x-rate bf16 DVE
  multiply, the sum-of-squares reduction is a ones-vector matmul, and
  silu is a batched ACT sigmoid + DVE multiply casting to fp8/bf16 on
  write.  The four branch norms share two fused sqrt/recip/broadcasts.
- Engines are in-order, so loops are software-pipelined: block g+1's
  normalize+silu is emitted during block g's GEMMs, each block's ss
  matmuls are deferred past the next block's GEMMs, and the first/last
  blocks around each norm barrier run per-tile (or per-half for the gate
  sigmoid/stt/tanh chain) to shorten restart and drain latency.
  Cross-phase weight prefetch tiles live in enclosing pool scopes so
  their DMAs do not inherit false WARs from reused SBUF regions.
- deter is read once as fp8 (branch-0 GEMM + L0 dg slice, resident) and
  once as bf16 (GRU mix operand, streamed during the gates phase);
  output is written bf16 and upcast on the host.
- Hardware-legality notes baked in: gpsimd(Pool) cannot touch PSUM and
  cannot run TensorScalarPtr; DoubleRow Ldweights rejects degenerate
  single-column stationary tiles; Sqrt lives in a different ACT table
  set than Sigmoid/Tanh, so table switches are pre-triggered by tiny
  warm-up ops off the critical path.
"""

import os
import sys
from contextlib import ExitStack

import numpy as np
import ml_dtypes as _ml

for _p in ("/opt/trn_rl_repo", "/opt/pypackages"):
    if os.path.isdir(_p) and _p not in sys.path:
        sys.path.insert(0, _p)

os.environ.setdefault("MYCRO_LOCAL_CACHE", "1")

import concourse.bass as bass  # noqa: E402
import concourse.bacc as bacc  # noqa: E402
import concourse.mybir as mybir  # noqa: E402
import concourse.tile as tile  # noqa: E402

# ---- problem constants (hardcoded; kernel.py must be self-contained) ----
P = 128
B = 4096
NCORES = 8
BC = B // NCORES  # 512 batch columns per core
DETER = 4096
STOCH = 1024
ACT_DIM = 32
DEMB = 16
HIDDEN = 512
BLOCKS = 8
OUT_B = DETER // BLOCKS  # 512
IN_B0 = 4 * HIDDEN + OUT_B  # 2560
EPS = 1e-4

ND = DETER // P       # 32 deter k/n tiles
NX = 4 * HIDDEN // P  # 16 x k tiles
WS = 64.0             # weight scale folded into rmsnorm / gate scales

# const-block column layout (single [P, C_NCOL] f32 DRAM input)
C_BX64 = 0            # 16: 64*(branch bias * gain)
C_BH064 = 16          # 32: 64*(bh0 * gh0)
C_BH164 = 48          # 32: 64*(bh1 * gh1)
C_BG = 80             # 96: bg (unscaled, sigmoid bias)
C_BG64 = 176          # 96: 64*bg (cand stt bias)
C_BGM1 = 272          # 96: bg - 1 (update sigmoid bias)
C_EPSH = 368          # 1: 4096*EPS
C_NEG1 = 369          # 1: -1.0 (update-gate bias)
C_NCOL = 370

f32 = mybir.dt.float32
f32r = mybir.dt.float32r
bf16 = mybir.dt.bfloat16
fp8 = mybir.dt.float8e4

# (L1 is the only bf16 GEMM; see fp8_sim3.py for the quantization study)

_PROG = {}


def _r(ap):
    return ap.bitcast(f32r)


def _build_program(zb):
    """Build the single-core SPMD Bass program (same on all 8 cores).

    zb: gate biases are all zero -> skip the bias K-row in the gate GEMM
    (saves a weight pair per m-tile and 3MB of DMA per core)."""
    AF = mybir.ActivationFunctionType
    Alu = mybir.AluOpType
    DR = mybir.MatmulPerfMode.DoubleRow
    nc = bacc.Bacc(trn_type="TRN2", target_bir_lowering=False, debug=False)

    def din(name, shape, dt=f32):
        return nc.dram_tensor(name, list(shape), dt, kind="ExternalInput").ap()

    dT8 = din("dT8", (DETER, BC), fp8)
    dTb = din("dTb", (DETER, BC), bf16)
    sT8 = din("sT8", (STOCH, BC), fp8)
    aT = din("aT", (ACT_DIM, BC))
    eT = din("eT", (DEMB, BC))
    W0p = din("W0p", (16, P, 2, HIDDEN), fp8)
    W1p = din("W1p", (P, 4, 2, HIDDEN), fp8)
    W2 = din("W2", (ACT_DIM, HIDDEN))
    W3 = din("W3", (DEMB, HIDDEN))
    Wh0x = din("Wh0x", (BLOCKS, P, 8, 2, OUT_B), fp8)
    Wh0d = din("Wh0d", (BLOCKS, P, 2, 2, OUT_B), fp8)
    Wh1 = din("Wh1", (BLOCKS, P, 4, OUT_B), bf16)
    NKP = 2 if zb else 3
    Wgp = din("Wgp", (BLOCKS, P, NKP, 2, 3 * OUT_B), fp8)
    cst = din("cst", (P, C_NCOL))
    cst8 = din("cst8", (P, 2 + 2 * BC), fp8)
    outT = nc.dram_tensor("outT", [DETER, BC], bf16,
                          kind="ExternalOutput").ap()

    with tile.TileContext(nc) as tc, ExitStack() as top:
        consts = top.enter_context(tc.tile_pool(name="consts", bufs=1))
        cst_sb = consts.tile([P, C_NCOL], f32)
        nc.sync.dma_start(out=_r(cst_sb), in_=_r(cst))
        cst8_sb = consts.tile([P, 2 + 2 * BC], fp8)
        nc.sync.dma_start(out=cst8_sb, in_=cst8)
        ones8 = cst8_sb[:, 0:2]
        ones_b16 = consts.tile([P, 1], bf16)
        nc.vector.memset(ones_b16, 1.0)
        onesp = cst8_sb[:, 2:2 + 2 * BC].rearrange("p (j b) -> p j b", j=2)
        bgm1 = cst_sb[:, C_BGM1:C_BGM1 + 96]
        epsh = cst_sb[:1, C_EPSH:C_EPSH + 1]
        neg1 = cst_sb[:, C_NEG1:C_NEG1 + 1]

        # resident regions
        mainp = top.enter_context(tc.tile_pool(name="mainp", bufs=1))
        main_sb = mainp.tile([P, ND, BC], bf16)   # pre-norm y64 / h0n
        dtbp = top.enter_context(tc.tile_pool(name="dtbp", bufs=1))
        dtb_sb = dtbp.tile([P, ND, BC], bf16)     # deter bf16 (L0 dg + mix)
        x8p = top.enter_context(tc.tile_pool(name="x8p", bufs=1))
        x8_sb = x8p.tile([P, NX, BC], fp8)        # branch outputs (L0 rhs)
        h1p = top.enter_context(tc.tile_pool(name="h1p", bufs=1))
        h1n8 = h1p.tile([P, ND, BC], fp8)         # L1 normalized (gates rhs)

        ysqp = top.enter_context(tc.tile_pool(name="ysqp", bufs=3))
        invp = top.enter_context(tc.tile_pool(name="invp", bufs=2))
        invbp = top.enter_context(tc.tile_pool(name="invbp", bufs=2))
        sigp = top.enter_context(tc.tile_pool(name="sigp", bufs=3))

        def act_warm(func, name):
            """Trigger an ACT table switch off the critical path."""
            t = invp.tile([1, 1], f32, tag="warm", name=f"warm_{name}")
            nc.scalar.activation(out=t, in_=epsh, func=func)

        def finish_norm(ss_flat, D, width, name):
            """invb = 1/(64*sqrt(ss/D + eps)), bf16, broadcast to all
            partitions.  ss_flat: [1, width*BC] (PSUM)."""
            sq = invp.tile([1, width * BC], f32, tag="sq", name=f"sq_{name}")
            nc.scalar.activation(out=sq, in_=ss_flat, func=AF.Sqrt,
                                 bias=epsh, scale=1.0 / D)
            act_warm(AF.Sigmoid, f"sg_{name}")  # reload hidden under recip
            inv1 = invp.tile([1, width * BC], bf16, tag="inv1",
                             name=f"inv1_{name}")
            with nc.allow_low_precision(reason="bf16 rstd broadcast"):
                nc.vector.reciprocal(inv1, sq)
            invb = invbp.tile([P, width * BC], bf16, tag="invb",
                              name=f"invb_{name}")
            nc.gpsimd.partition_broadcast(invb, inv1)
            return invb

        def norm_silu_unit(unit_y, invb, out_unit, name, per_tile=False,
                           mul_pool=False):
            """out_unit <- silu(unit_y * invb) = t*sigmoid(t), t=y*inv.
            per_tile=True pipelines at tile granularity (lower latency
            right after a norm barrier)."""
            mul_eng = nc.gpsimd if mul_pool else nc.vector
            for m in range(4):
                nc.vector.tensor_mul(unit_y[:, m, :], unit_y[:, m, :],
                                     invb[:, m * BC:(m + 1) * BC]
                                     if invb.shape[-1] == 4 * BC else invb)
            s = sigp.tile([P, 4, BC], bf16, tag="sig", name=f"sig_{name}")
            if per_tile:
                for m in range(4):
                    nc.scalar.activation(out=s[:, m, :], in_=unit_y[:, m, :],
                                         func=AF.Sigmoid)
                    mul_eng.tensor_mul(out_unit[:, m, :], unit_y[:, m, :],
                                       s[:, m, :])
            else:
                nc.scalar.activation(
                    out=s.rearrange("p a b -> p (a b)"),
                    in_=unit_y.rearrange("p a b -> p (a b)"), func=AF.Sigmoid)
                mul_eng.tensor_mul(
                    out_unit.rearrange("p a b -> p (a b)"),
                    unit_y.rearrange("p a b -> p (a b)"),
                    s.rearrange("p a b -> p (a b)"))

        # ============ phases A, L0, L1 (shared PSUM layout) ============
        with ExitStack() as ph_al:
            psum_acc = ph_al.enter_context(
                tc.tile_pool(name="pacc", bufs=7, space="PSUM"))
            psum_ss = ph_al.enter_context(
                tc.tile_pool(name="pss", bufs=1, space="PSUM"))
            d8pool = ph_al.enter_context(tc.tile_pool(name="d8r", bufs=1))
            d8sb = d8pool.tile([P, ND, BC], fp8)  # fp8 deter (br0 + L0 dg)

            def unit_post(unit_y, accs, b64col0, ss, ss_first, ss_last,
                          name, act_copy, sq_pool=False, presum=False):
                """copy accs (+64*bias) into unit_y (bf16) -- on ACT
                (Identity) when act_copy else DVE; squares (64y)^2 bf16.
                Returns a closure emitting the 4 ss ones-matmuls, so the
                caller can defer them past the next block's GEMMs (the PE
                is in-order; immediate ss would stall it on this block's
                elementwise post)."""
                for m in range(4):
                    bcol = cst_sb[:, b64col0 + m:b64col0 + m + 1]
                    if act_copy and (act_copy == 2 or m % 2 == 0):
                        nc.scalar.activation(out=unit_y[:, m, :],
                                             in_=accs[m], func=AF.Identity,
                                             bias=bcol)
                    else:
                        nc.vector.tensor_scalar_add(unit_y[:, m, :],
                                                    accs[m], bcol)
                ysq = ysqp.tile([P, 4, BC], bf16, tag="ysq",
                                name=f"ysq_{name}")
                if sq_pool:
                    for half in range(2):
                        seg = unit_y[:, 2 * half:2 * half + 2, :].rearrange(
                            "p a b -> p (a b)")
                        nc.gpsimd.tensor_mul(
                            ysq[:, 2 * half:2 * half + 2, :].rearrange(
                                "p a b -> p (a b)"), seg, seg)
                else:
                    for t in range(4):
                        nc.vector.tensor_mul(ysq[:, t, :], unit_y[:, t, :],
                                             unit_y[:, t, :])

                if presum:
                    # fold 4 tiles into 1 on the (idle) DVE; the PE then
                    # does a single ones-matmul instead of four
                    nc.vector.tensor_add(ysq[:, 0, :], ysq[:, 0, :],
                                         ysq[:, 1, :])
                    nc.vector.tensor_add(ysq[:, 2, :], ysq[:, 2, :],
                                         ysq[:, 3, :])
                    nc.vector.tensor_add(ysq[:, 0, :], ysq[:, 0, :],
                                         ysq[:, 2, :])

                    def emit_ss():
                        nc.tensor.matmul(ss, lhsT=ones_b16, rhs=ysq[:, 0, :],
                                         start=ss_first, stop=ss_last)
                else:
                    def emit_ss():
                        for t in range(4):
                            nc.tensor.matmul(
                                ss, lhsT=ones_b16, rhs=ysq[:, t, :],
                                start=(ss_first and t == 0),
                                stop=(ss_last and t == 3))
                return emit_ss

            # ---------------- phase A: four input branches ----------------
            with ExitStack() as ph_a:
                w0p_pool = ph_a.enter_context(
                    tc.tile_pool(name="w0p", bufs=8))
                sp = ph_a.enter_context(tc.tile_pool(name="sp", bufs=1))

                sT_sb = sp.tile([P, STOCH // P, BC], fp8)
                aT_sb = sp.tile([ACT_DIM, BC], f32)
                eT_sb = sp.tile([DEMB, BC], f32)
                an_sb = sp.tile([ACT_DIM, BC], f32)
                w3t = sp.tile([DEMB, HIDDEN], f32)
                w2t = sp.tile([ACT_DIM, HIDDEN], f32)
                w1t = sp.tile([P, 4, 2, HIDDEN], fp8)

                # the 4MB branch-0 stream is the phase-A long pole: first
                w0_slabs = []
                for c in range(8):
                    nc.sync.dma_start(
                        out=d8sb[:, 4 * c:4 * c + 4, :],
                        in_=dT8[512 * c:512 * (c + 1), :].rearrange(
                            "(s p) b -> p s b", p=P))
                    w0 = w0p_pool.tile([P, 2, 2, HIDDEN], fp8, tag="w0",
                                       name=f"w0_{c}")
                    nc.sync.dma_start(
                        out=w0, in_=W0p[2 * c:2 * c + 2].rearrange(
                            "s p j m -> p s j m"))
                    w0_slabs.append(w0)
                    if c == 0:
                        nc.sync.dma_start(out=_r(eT_sb), in_=_r(eT))
                        nc.sync.dma_start(out=_r(w3t), in_=_r(W3))
                        nc.sync.dma_start(out=aT_sb, in_=aT)
                        nc.sync.dma_start(out=_r(w2t), in_=_r(W2))
                    if c == 1:
                        nc.sync.dma_start(
                            out=sT_sb,
                            in_=sT8.rearrange("(s p) b -> p s b", p=P))
                        nc.sync.dma_start(out=w1t, in_=W1p)

                # action preprocess: a / max(|a|, 1)
                ab = sp.tile([ACT_DIM, BC], f32)
                nc.scalar.activation(out=ab, in_=aT_sb, func=AF.Abs)
                act_warm(AF.Sqrt, "br")
                nc.vector.tensor_scalar_max(ab, ab, 1.0)
                nc.vector.reciprocal(ab, ab)
                nc.vector.tensor_mul(_r(an_sb), aT_sb, ab)

                def branch_small(br, wt, rhs):
                    accs = []
                    for m in range(4):
                        acc = psum_acc.tile([P, BC], f32, tag="acc",
                                            name=f"acc_br{br}_{m}")
                        nc.tensor.matmul(acc,
                                         lhsT=_r(wt[:, m * P:(m + 1) * P]),
                                         rhs=_r(rhs), start=True, stop=True)
                        accs.append(acc)
                    return accs

                def branch_dr(br, npairs, wslab, rhs_pair):
                    accs = [psum_acc.tile([P, BC], f32, tag="acc",
                                          name=f"acc_br{br}_{m}")
                            for m in range(4)]
                    for kp in range(npairs):
                        w = wslab(kp)
                        rhs = rhs_pair(kp)
                        for m in range(4):
                            nc.tensor.matmul(
                                accs[m], lhsT=w[:, :, m * P:(m + 1) * P],
                                rhs=rhs, start=(kp == 0),
                                stop=(kp == npairs - 1), perf_mode=DR)
                    return accs

                # branches 1-3 share one fused norm (single sqrt/recip/
                # broadcast over 3 stacked ss slots); branch 0 -- whose GEMM
                # is gated on the big dT8/W0p DMA stream -- normalizes on
                # its own so the other three silus overlap that stream.
                ss32 = [psum_ss.tile([1, BC], f32, tag="ss", name=f"ss_b{b}")
                        for b in (3, 2)]

                def branch_post(br, accs, ss):
                    unit = main_sb[:, 4 * br:4 * br + 4, :]
                    unit_post(unit, accs, C_BX64 + 4 * br, ss,
                              True, True, f"br{br}", act_copy=2,
                              sq_pool=(br == 3))()

                branch_post(3, branch_small(3, w3t, eT_sb), ss32[0])
                branch_post(2, branch_small(2, w2t, an_sb), ss32[1])
                ssb1 = psum_ss.tile([1, BC], f32, tag="ss", name="ssb1")
                branch_post(1, branch_dr(
                    1, 4, lambda kp: w1t[:, kp, :, :],
                    lambda kp: sT_sb[:, 2 * kp:2 * kp + 2, :]), ssb1)
                for i, br in enumerate((3, 2)):
                    invb = finish_norm(ss32[i], HIDDEN, 1, f"br{br}")
                    norm_silu_unit(
                        main_sb[:, 4 * br:4 * br + 4, :], invb,
                        x8_sb[:, 4 * br:4 * br + 4, :], f"br{br}",
                        mul_pool=(br == 3))
                invb1b = finish_norm(ssb1, HIDDEN, 1, "br1")
                norm_silu_unit(main_sb[:, 4:8, :], invb1b,
                               x8_sb[:, 4:8, :], "br1")

                ssb0 = psum_ss.tile([1, BC], f32, tag="ss", name="ssb0")
                branch_post(0, branch_dr(
                    0, 16, lambda kp: w0_slabs[kp // 2][:, kp % 2, :, :],
                    lambda kp: d8sb[:, 2 * kp:2 * kp + 2, :]), ssb0)
                invb0b = finish_norm(ssb0, HIDDEN, 1, "br0")
                norm_silu_unit(main_sb[:, 0:4, :], invb0b,
                               x8_sb[:, 0:4, :], "br0", per_tile=True)

            # ---------- L0: BlockLinear(2560 -> 512/block) ----------
            with ExitStack() as ph_l:
                wh0xp = ph_l.enter_context(
                    tc.tile_pool(name="wh0xp", bufs=2))
                wh0dp = ph_l.enter_context(
                    tc.tile_pool(name="wh0dp", bufs=2))
                wh1p = ph_l.enter_context(tc.tile_pool(name="wh1p", bufs=3))

                ss0 = psum_ss.tile([1, BC], f32, tag="ss", name="ss_l0")
                pend0 = []
                for g in range(BLOCKS):
                    wd = wh0dp.tile([P, 2, 2, OUT_B], fp8, tag="wh0d",
                                    name=f"wh0d_{g}")
                    nc.sync.dma_start(out=wd, in_=Wh0d[g])
                    wx = wh0xp.tile([P, 8, 2, OUT_B], fp8, tag="wh0x",
                                    name=f"wh0x_{g}")
                    nc.sync.dma_start(out=wx, in_=Wh0x[g])
                    accs = [psum_acc.tile([P, BC], f32, tag="acc",
                                          name=f"acc_h0_{g}_{m}")
                            for m in range(4)]
                    # dg (fp8 DR) matmuls for all m first: they only need
                    # the resident fp8 deter + wd, so the PE can run them
                    # while the branch silu (x8) is still finishing
                    for m in range(4):
                        for kp in range(2):
                            nc.tensor.matmul(
                                accs[m],
                                lhsT=wd[:, kp, :, m * P:(m + 1) * P],
                                rhs=d8sb[:, 4 * g + 2 * kp:4 * g + 2 * kp + 2,
                                         :],
                                start=(kp == 0), stop=False, perf_mode=DR)
                    for m in range(4):
                        for kp in (2, 3, 4, 5, 6, 7, 0, 1):
                            nc.tensor.matmul(
                                accs[m], lhsT=wx[:, kp, :, m * P:(m + 1) * P],
                                rhs=x8_sb[:, 2 * kp:2 * kp + 2, :],
                                start=False, stop=(kp == 1), perf_mode=DR)
                    # deferred ss for the previous block: keeps the PE from
                    # stalling on this block's elementwise post
                    if pend0:
                        pend0.pop()()
                    pend0.append(unit_post(
                        main_sb[:, 4 * g:4 * g + 4, :], accs,
                        C_BH064 + 4 * g, ss0, g == 0, g == BLOCKS - 1,
                        f"l0_{g}", act_copy=True, sq_pool=(g % 2 == 0)))
                    if g == 5:
                        act_warm(AF.Sqrt, "l0")
                pend0.pop()()
                invb0 = finish_norm(ss0, DETER, 1, "l0")

                # --------- L1 (bf16), interleaved with the L0 norm ---------
                ss1 = psum_ss.tile([1, BC], f32, tag="ss", name="ss_l1")
                # h0n = silu(norm(h0)) in place; block 0 primed per-tile,
                # block g+1's silu is emitted during block g (in-order
                # engines would otherwise serialize consecutive blocks)
                norm_silu_unit(main_sb[:, 0:4, :], invb0,
                               main_sb[:, 0:4, :], "l1_0", per_tile=True)
                pend1 = []
                for g in range(BLOCKS):
                    unit = main_sb[:, 4 * g:4 * g + 4, :]
                    # stream the bf16 deter (GRU mix operand) here, where
                    # the DMA engines are otherwise idle
                    nc.sync.dma_start(
                        out=dtb_sb[:, 4 * g:4 * g + 4, :],
                        in_=dTb[512 * g:512 * (g + 1), :].rearrange(
                            "(s p) b -> p s b", p=P))
                    w1h = wh1p.tile([P, 4, OUT_B], bf16, tag="wh1",
                                    name=f"wh1_{g}")
                    nc.sync.dma_start(out=w1h, in_=Wh1[g])
                    accs = [psum_acc.tile([P, BC], f32, tag="acc",
                                          name=f"acc_h1_{g}_{m}")
                            for m in range(4)]
                    for m in range(4):
                        for kk in range(4):
                            nc.tensor.matmul(
                                accs[m], lhsT=w1h[:, kk, m * P:(m + 1) * P],
                                rhs=unit[:, kk, :],
                                start=(kk == 0), stop=(kk == 3))
                    if g + 1 < BLOCKS:
                        nxt = main_sb[:, 4 * (g + 1):4 * (g + 1) + 4, :]
                        norm_silu_unit(nxt, invb0, nxt, f"l1_{g + 1}")
                    if pend1:
                        pend1.pop()()
                    pend1.append(unit_post(
                        unit, accs, C_BH164 + 4 * g, ss1,
                        g == 0, g == BLOCKS - 1, f"l1_{g}",
                        act_copy=True, sq_pool=(g % 2 == 0)))
                    if g == 5:
                        act_warm(AF.Sqrt, "l1")
                pend1.pop()()
                invb1 = finish_norm(ss1, DETER, 1, "l1")

        # ------------- GRU gates + final mix (per block) -------------
        with ExitStack() as ph_g:
            gpsum = ph_g.enter_context(
                tc.tile_pool(name="gpsum", bufs=2, space="PSUM"))
            wgp = ph_g.enter_context(tc.tile_pool(name="wgpool", bufs=3))
            grup = ph_g.enter_context(tc.tile_pool(name="grup", bufs=3))

            norm_silu_unit(main_sb[:, 0:4, :], invb1, h1n8[:, 0:4, :],
                           "g0", per_tile=True)
            for g in range(BLOCKS):
                wg = wgp.tile([P, NKP, 2, 3 * OUT_B], fp8, tag="wg",
                              name=f"wg_{g}")
                nc.sync.dma_start(out=wg, in_=Wgp[g])
                r_sb = grup.tile([P, 4, BC], bf16, tag="r", name=f"r_{g}")
                c_sb = grup.tile([P, 4, BC], bf16, tag="c", name=f"c_{g}")
                u_sb = grup.tile([P, 4, BC], bf16, tag="u", name=f"u_{g}")

                def gate_group(grp):
                    """12 DoubleRow matmuls (2 data pairs + bias pair) into
                    a 4-bank PSUM group for gate third grp."""
                    acc4 = gpsum.tile([P, 4, BC], f32, tag="g4",
                                      name=f"acc_g{g}_{grp}")
                    for m in range(4):
                        mm = 4 * grp + m
                        for kp in range(NKP):
                            rhs = (onesp if kp == 2 else
                                   h1n8[:, 4 * g + 2 * kp:4 * g + 2 * kp + 2, :])
                            nc.tensor.matmul(
                                acc4[:, m, :],
                                lhsT=wg[:, kp, :, mm * P:(mm + 1) * P],
                                rhs=rhs, start=(kp == 0),
                                stop=(kp == NKP - 1), perf_mode=DR)
                    return acc4

                accr = gate_group(0)
                if g == BLOCKS - 1:
                    for m in range(4):
                        nc.scalar.activation(
                            out=r_sb[:, m, :], in_=accr[:, m, :],
                            func=AF.Sigmoid, scale=1.0 / WS)
                else:
                    for h in range(2):
                        nc.scalar.activation(
                            out=r_sb[:, 2 * h:2 * h + 2, :].rearrange(
                                "p a b -> p (a b)"),
                            in_=accr[:, 2 * h:2 * h + 2, :].rearrange(
                                "p a b -> p (a b)"),
                            func=AF.Sigmoid, scale=1.0 / WS)
                if g + 1 < BLOCKS:
                    nxt = main_sb[:, 4 * (g + 1):4 * (g + 1) + 4, :]
                    norm_silu_unit(nxt, invb1,
                                   h1n8[:, 4 * (g + 1):4 * (g + 1) + 4, :],
                                   f"g{g + 1}", mul_pool=False)
                accc = gate_group(1)
                if g == BLOCKS - 1:
                    for m in range(4):
                        nc.vector.scalar_tensor_tensor(
                            out=c_sb[:, m, :], in0=accc[:, m, :],
                            scalar=1.0 / WS, in1=r_sb[:, m, :],
                            op0=Alu.mult, op1=Alu.mult)
                        nc.scalar.activation(out=c_sb[:, m, :],
                                             in_=c_sb[:, m, :], func=AF.Tanh)
                else:
                    for h in range(2):
                        seg = c_sb[:, 2 * h:2 * h + 2, :].rearrange(
                            "p a b -> p (a b)")
                        nc.vector.scalar_tensor_tensor(
                            out=seg,
                            in0=accc[:, 2 * h:2 * h + 2, :].rearrange(
                                "p a b -> p (a b)"),
                            scalar=1.0 / WS,
                            in1=r_sb[:, 2 * h:2 * h + 2, :].rearrange(
                                "p a b -> p (a b)"),
                            op0=Alu.mult, op1=Alu.mult)
                        nc.scalar.activation(out=seg, in_=seg, func=AF.Tanh)
                accu = gate_group(2)
                if g == BLOCKS - 1:
                    for m in range(4):
                        nc.scalar.activation(
                            out=u_sb[:, m, :], in_=accu[:, m, :],
                            func=AF.Sigmoid, scale=1.0 / WS, bias=neg1)
                else:
                    nc.scalar.activation(
                        out=u_sb.rearrange("p a b -> p (a b)"),
                        in_=accu.rearrange("p a b -> p (a b)"),
                        func=AF.Sigmoid, scale=1.0 / WS, bias=neg1)

                if g < BLOCKS - 1:
                    cflat = c_sb.rearrange("p a b -> p (a b)")
                    dunit = dtb_sb[:, 4 * g:4 * g + 4, :].rearrange(
                        "p a b -> p (a b)")
                    uflat = u_sb.rearrange("p a b -> p (a b)")
                    # out = d + u*(c-d), in place in c_sb: sub alternates
                    # Pool/DVE by block parity, mul+add on DVE
                    sub_eng = nc.vector
                    sub_eng.tensor_sub(cflat, cflat, dunit)
                    nc.vector.tensor_mul(cflat, uflat, cflat)
                    nc.vector.tensor_add(cflat, dunit, cflat)
                    nc.sync.dma_start(
                        out=outT[512 * g:512 * (g + 1), :].rearrange(
                            "(s p) b -> p s b", p=P),
                        in_=c_sb)
                else:
                    # last block: per-tile mix so the out DMA overlaps
                    for m in range(4):
                        ct = c_sb[:, m, :]
                        dt_ = dtb_sb[:, 4 * g + m, :]
                        nc.vector.tensor_sub(ct, ct, dt_)
                        nc.vector.tensor_mul(ct, u_sb[:, m, :], ct)
                        nc.vector.tensor_add(ct, dt_, ct)
                        nc.sync.dma_start(
                            out=outT[512 * g + P * m:512 * g + P * (m + 1),
                                     :],
                            in_=ct)

    nc.compile()
    return nc


def _get_program(zb=True):
    if zb not in _PROG:
        _PROG[zb] = _build_program(zb)
    return _PROG[zb]


FP8NP = _ml.float8_e4m3


def _drpack(W, dt):
    """[K, M] -> [K//256, 128, 2, M] DoubleRow-packed, cast to dt."""
    K, M = W.shape
    return np.ascontiguousarray(
        W.reshape(K // 256, 2, P, M).transpose(0, 2, 1, 3)).astype(dt)


def _kpack(W, dt):
    """[K, M] -> [128, K//128, M] (plain k-tiled lhsT), cast to dt."""
    K, M = W.shape
    return np.ascontiguousarray(
        W.reshape(K // P, P, M).transpose(1, 0, 2)).astype(dt)


def _make_const_block(inputs):
    f = lambda a: np.asarray(a, dtype=np.float32)
    cst = np.zeros((P, C_NCOL), dtype=np.float32)
    cst[:, C_BX64:C_BX64 + 16] = WS * np.stack(
        [f(inputs[b]) * f(inputs[g]) for b, g in
         (("b0", "g0"), ("b1", "g1"), ("b2", "g2"), ("b3", "g3"))]
    ).reshape(16, P).T
    cst[:, C_BH064:C_BH064 + 32] = WS * (
        f(inputs["bh0"]) * f(inputs["gh0"])).reshape(32, P).T
    bgt = f(inputs["bg"]).reshape(96, P).T
    cst[:, C_BH164:C_BH164 + 32] = WS * (
        f(inputs["bh1"]) * f(inputs["gh1"])).reshape(32, P).T
    cst[:, C_BG:C_BG + 96] = bgt
    cst[:, C_BG64:C_BG64 + 96] = WS * bgt
    cst[:, C_BGM1:C_BGM1 + 96] = bgt - 1.0
    cst[:, C_EPSH] = WS * WS * EPS
    cst[:, C_NEG1] = -1.0
    return cst


def _prep_inputs(inputs, zb=True):
    """Host-side shard + transpose + quantized weight packing."""
    f = lambda a: np.ascontiguousarray(np.asarray(a), dtype=np.float32)
    stoch = f(inputs["stoch"]).reshape(B, -1)
    deter = f(inputs["deter"])
    action = f(inputs["action"])
    d_emb = f(inputs["d_emb"])

    g0, g1 = f(inputs["g0"]), f(inputs["g1"])
    g2, g3 = f(inputs["g2"]), f(inputs["g3"])
    gh0, gh1 = f(inputs["gh0"]), f(inputs["gh1"])

    W0 = WS * f(inputs["W0"]) * g0
    W1 = WS * f(inputs["W1"]) * g1
    Wh0 = WS * f(inputs["Wh0"]) * gh0.reshape(BLOCKS, 1, OUT_B)
    Wh1 = WS * f(inputs["Wh1"]) * gh1.reshape(BLOCKS, 1, OUT_B)
    Wg = WS * f(inputs["Wg"])

    wh0d = np.ascontiguousarray(np.stack(
        [_drpack(Wh0[g, :OUT_B], FP8NP) for g in range(BLOCKS)]
    ).transpose(0, 2, 1, 3, 4))  # [B, P, 2, 2, M]
    wh1 = np.stack([_kpack(Wh1[g], _ml.bfloat16) for g in range(BLOCKS)])

    bg = f(inputs["bg"])  # [3*DETER], block g segment [1536g:1536(g+1)]
    nkp = 2 if zb else 3
    wgp = np.zeros((BLOCKS, nkp, P, 2, 3 * OUT_B), dtype=FP8NP)
    for g in range(BLOCKS):
        wgp[g, :2] = _drpack(Wg[g], FP8NP)
        if not zb:
            wgp[g, 2, 0, 0, :] = (
                WS * bg[1536 * g:1536 * (g + 1)]).astype(FP8NP)

    cst8 = np.zeros((P, 2 + 2 * BC), dtype=FP8NP)
    cst8[:, 0:2] = 1.0
    cst8[0, 2:2 + BC] = 1.0  # bias-row rhs: partition 0, j=0 ones
    shared = {
        "W0p": _drpack(W0, FP8NP),
        "W1p": np.ascontiguousarray(
            _drpack(W1, FP8NP).transpose(1, 0, 2, 3)),  # [P, 4, 2, M]
        "W2": (WS * f(inputs["W2"]) * g2).astype(np.float32),
        "W3": (WS * f(inputs["W3"]) * g3).astype(np.float32),
        "Wh0x": np.stack([_drpack(Wh0[g, OUT_B:], FP8NP)
                          for g in range(BLOCKS)]),
        "Wh0d": wh0d,
        "Wh1": wh1,
        "Wgp": np.ascontiguousarray(wgp.transpose(0, 2, 1, 3, 4)),
        "cst": _make_const_block(inputs),
        "cst8": cst8,
    }
    # Wh0x packed as [B, pairs, P, 2, M] -> want [B, P, pairs, 2, M]
    shared["Wh0x"] = np.ascontiguousarray(
        shared["Wh0x"].transpose(0, 2, 1, 3, 4))
    # W0p stays [16, P, 2, M] (indexed by pair in the DMA loop)

    in_maps = []
    for c in range(NCORES):
        sl = slice(c * BC, (c + 1) * BC)
        m = dict(shared)
        dT = np.ascontiguousarray(deter[sl].T)
        m["dT8"] = dT.astype(FP8NP)
        m["dTb"] = dT.astype(_ml.bfloat16)
        m["sT8"] = np.ascontiguousarray(stoch[sl].T).astype(FP8NP)
        m["aT"] = np.ascontiguousarray(action[sl].T)
        m["eT"] = np.ascontiguousarray(d_emb[sl].T)
        in_maps.append(m)
    return in_maps


def _gate_bias_zero(inputs):
    return not np.any(np.asarray(inputs["bg"]))


def _run(inputs, trace=False):
    from concourse import bass_utils
    zb = _gate_bias_zero(inputs)
    nc = _get_program(zb)
    in_maps = _prep_inputs(inputs, zb)
    res = bass_utils.run_bass_kernel_spmd(
        nc, in_maps, core_ids=list(range(NCORES)), trace=trace)
    out = np.empty((B, DETER), dtype=np.float32)
    for c in range(NCORES):
        out[c * BC:(c + 1) * BC, :] = \
            np.asarray(res.results[c]["outT"]).astype(np.float32).T
    return out, res.exec_time_ns


def kernel(**inputs):
    out, _ = _run(inputs, trace=False)
    return out


# ---------------------------------------------------------------------------
# benchmarking helper (test-only; the grading path is kernel() above)
# ---------------------------------------------------------------------------

def _bench_generic(nc, in_maps, iters, n_cores=None):
    """Time repeated device executions with device-resident inputs."""
    import time
    import jax
    import concourse.mybir as mybir
    from jax.sharding import Mesh, NamedSharding, PartitionSpec
    from jax.experimental.shard_map import shard_map
    from concourse import bass2jax

    bass2jax.install_neuronx_cc_hook()
    if n_cores is None:
        n_cores = len(in_maps)

    in_names, out_names, out_avals = [], [], []
    for alloc in nc.m.functions[0].allocations:
        if not isinstance(alloc, mybir.MemoryLocationSet):
            continue
        name = alloc.memorylocations[0].name
        pid_name = (nc.partition_id_tensor.name
                    if nc.partition_id_tensor else None)
        if alloc.kind == "ExternalInput":
            if name != pid_name:
                in_names.append(name)
        elif alloc.kind == "ExternalOutput":
            out_names.append(name)
            out_avals.append(jax.core.ShapedArray(
                tuple(alloc.tensor_shape), mybir.dt.np(alloc.dtype)))
    n_params = len(in_names)

    pid_name = nc.partition_id_tensor.name if nc.partition_id_tensor else None
    bind_names = in_names + out_names + ([pid_name] if pid_name else [])

    def _body(*args):
        operands = list(args)
        if pid_name:
            operands.append(bass2jax.partition_id_tensor())
        outs = bass2jax._bass_exec_p.bind(
            *operands,
            out_avals=tuple(out_avals),
            in_names=tuple(bind_names),
            out_names=tuple(out_names),
            lowering_input_output_aliases=(),
            sim_require_finite=True,
            sim_require_nnan=True,
            nc=nc,
        )
        return tuple(outs)

    devices = jax.devices()[:n_cores]
    mesh = Mesh(np.asarray(devices), ("core",))
    nshard = NamedSharding(mesh, PartitionSpec("core"))
    sharded = jax.jit(
        shard_map(_body, mesh=mesh,
                  in_specs=(PartitionSpec("core"),) * (n_params + len(out_names)),
                  out_specs=(PartitionSpec("core"),) * len(out_names),
                  check_rep=False),
        keep_unused=True)

    concat_in = [
        jax.device_put(
            np.concatenate([np.asarray(in_maps[c][nm]) for c in range(n_cores)],
                           axis=0), nshard)
        for nm in in_names]
    concat_zeros = [
        jax.device_put(
            np.zeros((n_cores * a.shape[0], *a.shape[1:]), a.dtype), nshard)
        for a in out_avals]

    outs = sharded(*concat_in, *concat_zeros)
    jax.block_until_ready(outs)

    BATCH = 6
    diffs = []
    for _ in range(iters):
        t0 = time.perf_counter()
        outs = sharded(*concat_in, *concat_zeros)
        jax.block_until_ready(outs)
        t1 = time.perf_counter()
        for _ in range(BATCH):
            outs = sharded(*concat_in, *concat_zeros)
        jax.block_until_ready(outs)
        t2 = time.perf_counter()
        diffs.append((t2 - t1) - (t1 - t0))
    diffs.sort()
    per_iter_ns = diffs[len(diffs) // 2] / (BATCH - 1) * 1e9
    return outs, per_iter_ns


_TINY = None


def _tiny_program():
    """Near-noop program with the SAME input/output signature, to measure
    axon dispatch overhead differentially."""
    global _TINY
    if _TINY is None:
        nc = bacc.Bacc(trn_type="TRN2", target_bir_lowering=False, debug=False)
        d = {"dT8": ((DETER, BC), fp8), "dTb": ((DETER, BC), bf16),
             "sT8": ((STOCH, BC), fp8), "aT": ((ACT_DIM, BC), f32),
             "eT": ((DEMB, BC), f32), "W0p": ((16, P, 2, HIDDEN), fp8),
             "W1p": ((P, 4, 2, HIDDEN), fp8), "W2": ((ACT_DIM, HIDDEN), f32),
             "W3": ((DEMB, HIDDEN), f32),
             "Wh0x": ((BLOCKS, P, 8, 2, OUT_B), fp8),
             "Wh0d": ((BLOCKS, P, 2, 2, OUT_B), fp8),
             "Wh1": ((BLOCKS, P, 4, OUT_B), bf16),
             "Wgp": ((BLOCKS, P, 2, 2, 3 * OUT_B), fp8),
             "cst": ((P, C_NCOL), f32), "cst8": ((P, 2 + 2 * BC), fp8)}
        aps = {k: nc.dram_tensor(k, list(s), dt, kind="ExternalInput").ap()
               for k, (s, dt) in d.items()}
        outT = nc.dram_tensor("outT", [DETER, BC], bf16,
                              kind="ExternalOutput").ap()
        with tile.TileContext(nc) as tc:
            with tc.tile_pool(name="t", bufs=2) as pool:
                t = pool.tile([P, 4, BC], bf16)
                nc.sync.dma_start(
                    out=t, in_=aps["dTb"][:512, :].rearrange(
                        "(s p) b -> p s b", p=P))
                for g in range(BLOCKS):
                    nc.sync.dma_start(
                        out=outT[512 * g:512 * (g + 1), :].rearrange(
                            "(s p) b -> p s b", p=P),
                        in_=t)
        nc.compile()
        _TINY = nc
    return _TINY


def _bench_overhead(inputs, iters=20):
    nc = _tiny_program()
    in_maps = _prep_inputs(inputs, True)
    _, t = _bench_generic(nc, in_maps, iters)
    return t


def _bench(inputs, iters=20):
    zb = _gate_bias_zero(inputs)
    nc = _get_program(zb)
    in_maps = _prep_inputs(inputs, zb)
    outs, per_iter_ns = _bench_generic(nc, in_maps, iters)
    res = np.asarray(outs[0]).reshape(NCORES, DETER, BC)
    out = np.empty((B, DETER), dtype=np.float32)
    for c in range(NCORES):
        out[c * BC:(c + 1) * BC, :] = res[c].astype(np.float32).T
    return out, per_iter_ns
